# revision 52
# baseline (speedup 1.0000x reference)
"""Trainium2 Bass kernel for MultiHeadAttentionRoPE.

Problem (hardcoded): B=2, S=2048, D=1024, H=16 heads, Dh=64, fp32 I/O.
    qkv = x @ w_qkv ; q,k -> RoPE ; causal attention ; out = ctx @ w_proj

Sharding: core = (batch, head-quad). Each of the 8 cores handles one of
the 2 batches and 4 of the 16 heads: it reads its batch's x (bf16,
transposed on host), its 4-head slice of w_qkv/w_proj, computes causal
attention for those heads and a *partial* projection output [S, D]; the
host sums the 4 partials per batch (the gather step of row-parallel TP).

All matmul operands are bf16 (fp32 PSUM accumulation); rel-err vs the
fp32 reference is ~6e-3 (tolerance 2e-2).

v2: 512-token pipeline. Everything is tiled in 512-token groups (vs 1024
in v1) and emission is software-pipelined so the in-order PE queue always
has work while ACT chews the exp stream (ACT-exp is within ~25% of the PE
roofline, so attention rounds alone starve the PE):

  s1(0,0) | a(0,0)+s1(0,1) | a(0,1)+s1(1,0) | a(1,0)+s1(1,1) |
  a(1,1)+s1(2,0)+proj(0) | a(2,0)+s1(2,1) | a(2,1)+s1(3,0)+proj(1) |
  a(3,0)+s1(3,1) | a(3,1)+proj(2) | proj(3)

Each attention call's rounds are interleaved with "filler" units (stage1
psq chunks, RoPE, projection halves) pulled from the next pipeline stage.

PSUM (8 banks of 2KB): scores 2x[128,2,512]f32 (both heads of a round in
one tile so each round runs ONE exp over 1024 cols -- the ACT per-inst
PSUM-access overhead is ~143ns and ACT is within ~25% of PE), pv
2x[65,512]f32, filler pool 2x[128,512]f32 for stage1 psq / v-transpose
scratch / projection column-halves.

Per-core layout (as v1):
  - x fed transposed (d on partitions); head pairs packed per 128-partition
    tile (h_even rows 0:64, h_odd 64:128) in qT/kT/ctxT.
  - RoPE: rotate-half via partition-swapping SBUF->SBUF DMAs on a bf16
    staging tile, sin sign-folded on host; combine on DVE (bf16 2x mode).
  - scores transposed (keys on partitions, queries free); exp on ACT;
    denominator from a ones-column appended per head in the vt blocks
    (ones written by a strided Pool memset, not a DRAM load).
  - causal: key-blocks above the diagonal skipped; diagonal 128-blocks get
    a triangular 0/1 mask multiply (DVE, bf16 2x).
  - normalize: den row psum->sbuf (DVE), fast approx reciprocal (DVE),
    partition-broadcast (Pool), applied in the psum->sbuf ctx downcast.
  - projection per 128-token block in 512-col halves; partials staged in
    sbuf and stored per-half on the SP queue.
"""

import functools
import os
import sys

import numpy as np

sys.path.insert(0, "/opt/trn_rl_repo")

# ---- problem constants (must match reference.py) ----
B = 2
S = 2048
D = 1024
H = 16
Dh = 64
N_CORES = 8
HPC = 4                     # heads per core
KC = D // 128               # contraction chunks = 8
TCH = 512                   # token chunk (group / query block)
NG = S // TCH               # 4 groups
KPG = TCH // 128            # key blocks per group = 4
NKB = S // 128              # 16 key blocks
ROPE_BASE = 10000.0
SCALE = 1.0 / 8.0           # 1/sqrt(Dh)


def _build_program(loop_n=1, phases="all", opts=""):
    import concourse.bass as bass
    opts = set(opts.split(",")) if opts else set()
    import concourse.mybir as mybir
    import concourse.tile as tile
    from concourse import bacc
    from contextlib import ExitStack

    FP = mybir.dt.float32
    BF = mybir.dt.bfloat16
    FPR = mybir.dt.float32r
    EXP = mybir.ActivationFunctionType.Exp
    R = lambda ap: ap.bitcast(FPR)

    nc = bacc.Bacc("TRN2", target_bir_lowering=False, debug=False)

    xt_d = nc.dram_tensor("xt", [KC, 128, S], BF, kind="ExternalInput").ap()
    # wqk half-major: [half, kc, 128, q|k|v x 128] so stage1(0,0) only waits
    # on the half-0 blocks at startup
    wqk_d = nc.dram_tensor("wqk", [2, KC, 128, 3 * 128], BF, kind="ExternalInput").ap()
    wproj_d = nc.dram_tensor("wproj", [2, 128, D], BF, kind="ExternalInput").ap()
    cos_d = nc.dram_tensor("cost", [128, S], BF, kind="ExternalInput").ap()
    sin_d = nc.dram_tensor("sint", [128, S], BF, kind="ExternalInput").ap()
    tri_d = nc.dram_tensor("tri", [128, 128], BF, kind="ExternalInput").ap()
    ones_d = nc.dram_tensor("onesc", [128, NKB, 65 * HPC], BF, kind="ExternalInput").ap()
    onesr_d = nc.dram_tensor("onesr", [1, 64], FP, kind="ExternalInput").ap()
    ident_d = nc.dram_tensor("ident", [128, 128], BF, kind="ExternalInput").ap()
    out_d = nc.dram_tensor("out", [S, D], BF, kind="ExternalOutput").ap()
    dbg = phases == "dbg"
    if dbg:
        qt_dbg = nc.dram_tensor("qt_dbg", [2, 128, S], BF, kind="ExternalOutput").ap()
        kt_dbg = nc.dram_tensor("kt_dbg", [2, 128, S], BF, kind="ExternalOutput").ap()
        vt_dbg = nc.dram_tensor("vt_dbg", [128, NKB, 65 * HPC], BF, kind="ExternalOutput").ap()
        ctx_dbg = nc.dram_tensor("ctx_dbg", [2, 128, S], BF, kind="ExternalOutput").ap()

    with tile.TileContext(nc) as tc, ExitStack() as ctx:
        consts = ctx.enter_context(tc.tile_pool(name="consts", bufs=1))
        store = ctx.enter_context(tc.tile_pool(name="store", bufs=1))
        xt_pool = ctx.enter_context(tc.tile_pool(name="xt_pool", bufs=3))
        rt_pool = ctx.enter_context(tc.tile_pool(name="rt_pool", bufs=2))
        p_pool = ctx.enter_context(tc.tile_pool(name="p_pool", bufs=6))
        nrm_pool = ctx.enter_context(tc.tile_pool(name="nrm_pool", bufs=2))
        ob_pool = ctx.enter_context(tc.tile_pool(name="ob_pool", bufs=4))
        # PSUM: scores 3 banks, pv 3 banks, filler 2 banks (psq / v-transpose
        # scratch / proj halves).
        ps_s = ctx.enter_context(tc.tile_pool(name="ps_s", bufs=2, space="PSUM"))
        ps_v = ctx.enter_context(tc.tile_pool(name="ps_v", bufs=2, space="PSUM"))
        ps_f = ctx.enter_context(tc.tile_pool(name="ps_f", bufs=2, space="PSUM"))

        # ---- constants ----
        wqk_sb = consts.tile([128, 2, KC, 3 * 128], BF, name="wqk_sb")
        ident_sb = consts.tile([128, 128], BF, name="ident_sb")
        wproj_sb = consts.tile([128, 2, D], BF, name="wproj_sb")
        cos_sb = consts.tile([128, S], BF, name="cos_sb")
        sin_sb = consts.tile([128, S], BF, name="sin_sb")
        tri_sb = consts.tile([128, 128], BF, name="tri_sb")
        onesr_sb = consts.tile([1, 64], FP, name="onesr_sb")

        # ---- persistent per-core storage ----
        # qT/kT chunk c holds heads (2c, 2c+1): rows [h dh0..63 | h' dh0..63]
        qT = {c: store.tile([128, S], BF, name=f"qT_{c}", tag=f"qT_{c}") for c in range(2)}
        kT = {c: store.tile([128, S], BF, name=f"kT_{c}", tag=f"kT_{c}") for c in range(2)}
        ctxT = {c: store.tile([128, S], BF, name=f"ctxT_{c}", tag=f"ctxT_{c}") for c in range(2)}
        # vt: per key block kb, cols [v_h0 |1| v_h1 |1| v_h2 |1| v_h3 |1]
        vt = store.tile([128, NKB, 65 * HPC], BF, name="vt", tag="vt")

        def late_consts():
            with tc.tile_wait_until(0.003):
                # group-0 RoPE table slices first (on separate queues): the
                # startup RoPE only needs cols 0:TCH
                nc.sync.dma_start(cos_sb[:, 0:TCH], cos_d[:, 0:TCH])
                nc.scalar.dma_start(sin_sb[:, 0:TCH], sin_d[:, 0:TCH])
                nc.sync.dma_start(ident_sb, ident_d)
                nc.sync.dma_start(tri_sb, tri_d)
                nc.sync.dma_start(R(onesr_sb), R(onesr_d))
            with tc.tile_wait_until(0.012):
                nc.sync.dma_start(cos_sb[:, TCH:S], cos_d[:, TCH:S])
                nc.sync.dma_start(sin_sb[:, TCH:S], sin_d[:, TCH:S])
            with tc.tile_wait_until(0.015):
                for kc in range(KC):
                    eng = nc.sync if kc % 2 == 0 else nc.gpsimd
                    eng.dma_start(wqk_sb[:, 1, kc, :], wqk_d[1, kc])
            with tc.tile_wait_until(0.006):
                # whole-tile ones init: the interleave copies overwrite the v
                # columns; col 64 of each 65-block stays 1 (the denominator
                # column). Full-tile write gives unambiguous ordering vs the
                # PV matmul reads (a strided column DMA racing with the
                # interleave writes corrupted adjacent values on HW in v1).
                nc.sync.dma_start(vt, ones_d)
            with tc.tile_wait_until(0.030):
                for i in range(2):
                    nc.gpsimd.dma_start(wproj_sb[:, i, :], wproj_d[i])

        xtiles = {}

        def prefetch(g):
            """Issue group g's x loads ~a phase before stage1(g) runs."""
            xtile = xt_pool.tile([128, KC, TCH], BF, name="xtile", tag="xt")
            xtiles[g] = xtile
            tsl = slice(g * TCH, (g + 1) * TCH)
            for kc in range(KC):
                nc.gpsimd.dma_start(xtile[:, kc, :], xt_d[kc, :, tsl])

        def stage1(g, half):
            """QKV^T projection + RoPE + v natural layout for token group g
            (512 tokens), head pair `half`. Yields between units so the
            driver can weave it into attention rounds."""
            tsl = slice(g * TCH, (g + 1) * TCH)
            if half == 0 and g == 0:
                xtile = xt_pool.tile([128, KC, TCH], BF, name="xtile", tag="xt")
                xtiles[g] = xtile
                for kc in range(KC):
                    # startup-critical: spread x + half-0 wqk across the
                    # SP/Pool/ACT queues, each kc's pair on two queues
                    q3 = (nc.sync, nc.gpsimd, nc.scalar)
                    q3[kc % 3].dma_start(xtile[:, kc, :], xt_d[kc, :, tsl])
                    q3[(kc + 1) % 3].dma_start(wqk_sb[:, 0, kc, :], wqk_d[0, kc])
                late_consts()
            xtile = xtiles[g]
            rtile = rt_pool.tile([128, 2, TCH], BF, name="rtile", tag="rt")
            qs = rt_pool.tile([128, 2, TCH], BF, name="qs", tag="qs")
            # i: 0 = q, 1 = k, 2 = v; blk indexes the half's [q|k|v] blocks.
            # q first: its RoPE chain gates the next phase's first scores.
            for i, blk in enumerate((0, 1, 2)):
                psq = ps_f.tile([128, TCH], FP, name="psq", tag="ps_f")
                for kc in range(KC):
                    nc.tensor.matmul(
                        psq,
                        lhsT=wqk_sb[:, half, kc, blk * 128:(blk + 1) * 128],
                        rhs=xtile[:, kc, :],
                        start=(kc == 0),
                        stop=(kc == KC - 1),
                    )
                    if kc == 3:
                        yield
                yield
                if i < 2:
                    # stage q/k in bf16 for the rotate-half partition swap.
                    # g==0: ACT (idle at startup; DVE runs the RoPE chains
                    # that gate the first scores). g>0: DVE (ACT runs exps).
                    if g == 0:
                        nc.scalar.copy(rtile[:, i, :], psq)
                    else:
                        nc.vector.tensor_copy(out=rtile[:, i, :], in_=psq)
                    # issue this operand's rotate-half partition swaps
                    # immediately (k's swaps overlap q's projection),
                    # spread across the SP/Pool/ACT queues
                    qeng = (
                        (nc.sync, nc.gpsimd, nc.scalar, nc.sync) if g == 0
                        else (nc.sync, nc.gpsimd, nc.sync, nc.gpsimd)
                    )
                    for j, (d0, s0) in enumerate(
                        ((0, 32), (32, 0), (64, 96), (96, 64))
                    ):
                        qeng[j].dma_start(
                            qs[d0:d0 + 32, i, :], rtile[s0:s0 + 32, i, :]
                        )
                    if i == 1:
                        yield
                        # q first: the first score matmul needs all of qT but
                        # only the first key block of kT
                        order = ((0, qT[half]), (1, kT[half]))
                        for j, dest in order:
                            t1 = rt_pool.tile([128, TCH], BF, name="t1", tag="t1")
                            nc.vector.tensor_mul(out=t1, in0=qs[:, j, :], in1=sin_sb[:, tsl])
                            nc.vector.tensor_mul(out=dest[:, tsl], in0=rtile[:, j, :], in1=cos_sb[:, tsl])
                            nc.vector.tensor_add(out=dest[:, tsl], in0=dest[:, tsl], in1=t1)
                            yield
                else:
                    # v -> natural layout via PE transposes
                    vts = rt_pool.tile([128, TCH], BF, name="vts", tag="vts")
                    if g == 0:
                        nc.scalar.copy(vts, psq)
                    else:
                        nc.vector.tensor_copy(out=vts, in_=psq)
                    yield
                    pv4 = ps_f.tile([128, KPG, 128], BF, name="pv4", tag="ps_f")
                    for sc in range(KPG):
                        nc.tensor.transpose(
                            pv4[:, sc, :], vts[:, sc * 128:(sc + 1) * 128], ident_sb
                        )
                    yield
                    # interleave into vt: head pair (2*half, 2*half+1)
                    v2 = vt[:, g * KPG:(g + 1) * KPG, :].rearrange(
                        "p k (a c) -> p k a c", c=65
                    )[:, :, 2 * half:2 * half + 2, 0:64]
                    s2 = pv4.rearrange("p k (a c) -> p k a c", c=64)
                    nc.vector.tensor_copy(out=v2, in_=s2)
                    yield

        def attention(qb, r):
            """Causal attention for 512-token query block qb, head pair r.
            Yields once per key-block round (the weave points)."""
            q0 = qb * TCH
            nkc = KPG * qb + KPG
            hs = (2 * r, 2 * r + 1)

            def score_exp(kc):
                """Both heads' scores into one 2-bank psum tile -> ONE exp
                (halves the ACT per-instruction PSUM-access overhead)."""
                off = max(0, (kc - KPG * qb) * 128)
                c = r
                ps = ps_s.tile([128, 2, TCH], FP, name="ps", tag="ps_s")
                for hi in range(2):
                    hb = hi * 64
                    nc.tensor.matmul(
                        ps[:, hi, off:TCH],
                        lhsT=kT[c][hb:hb + 64, kc * 128:(kc + 1) * 128],
                        rhs=qT[c][hb:hb + 64, q0 + off:q0 + TCH],
                        start=True,
                        stop=True,
                    )
                p = p_pool.tile([128, 2, TCH], BF, name="p", tag="p")
                nc.scalar.activation(p[:, :, off:TCH], ps[:, :, off:TCH], EXP, scale=SCALE)
                if kc >= KPG * qb:  # diagonal band: triangular mask
                    # DVE: tiny (bf16 2x) and on the exp->PV critical path;
                    # Pool would queue it behind DMA descriptor generation
                    eng = nc.gpsimd if "poolmask" in opts else nc.vector
                    for hi in range(2):
                        eng.tensor_mul(
                            out=p[:, hi, off:off + 128],
                            in0=p[:, hi, off:off + 128],
                            in1=tri_sb,
                        )
                return p

            pv = {h: ps_v.tile([65, TCH], FP, name=f"ppv{h}", tag="ps_v") for h in hs}

            def pv_mm(kc, p):
                off = max(0, (kc - KPG * qb) * 128)
                for hi, h in enumerate(hs):
                    nc.tensor.matmul(
                        pv[h][:, off:TCH],
                        lhsT=vt[:, kc, 65 * h:65 * h + 65],
                        rhs=p[:, hi, off:TCH],
                        start=(kc == 0),
                        stop=(kc == nkc - 1),
                    )

            pk = {}
            for kc in range(nkc):
                pk[kc] = score_exp(kc)
                if kc >= 2:
                    pv_mm(kc - 2, pk.pop(kc - 2))
                yield
            for kc in (nkc - 2, nkc - 1):
                if kc >= 0:
                    pv_mm(kc, pk.pop(kc))
            # normalize: ctx rows 0..63 per head, denominator row 64.
            def normalize(h, c0, c1):
                hb = (h % 2) * 64
                c = h // 2
                n = c1 - c0
                den = nrm_pool.tile([1, TCH], FP, name="den", tag="den")
                dn = den[:, 0:n]
                nc.vector.tensor_copy(out=R(dn), in_=pv[h][64:65, c0:c1])
                rcp = nrm_pool.tile([1, TCH], FP, name="rcp", tag="rcp")
                rp = rcp[:, 0:n]
                nc.vector.reciprocal_approx_fast(out=rp, in_=dn)
                rcb = nrm_pool.tile([64, TCH], FP, name="rcb", tag="rcb")
                rb = rcb[:, 0:n]
                nc.gpsimd.partition_broadcast(rb, rp, channels=64)
                nc.vector.tensor_mul(
                    out=ctxT[c][hb:hb + 64, q0 + c0:q0 + c1],
                    in0=pv[h][0:64, c0:c1], in1=rb,
                )

            if qb == NG - 1:
                # 128-col chunks so proj(3)'s first token block can start
                # after the first chunk instead of the full chain
                for ch in range(KPG):
                    for h in hs:
                        normalize(h, ch * 128, (ch + 1) * 128)
                    yield
            else:
                for h in hs:
                    normalize(h, 0, TCH)
                    yield

        def proj(qb):
            """Projection for query block qb (4 token blocks of 128), in
            512-col halves so each occupies one filler PSUM bank. Drains go
            to DVE while attention exps overlap; the final block's drain is
            split DVE/ACT with half-stores on two queues to shorten the
            kernel tail."""
            last = qb == NG - 1
            for j in range(KPG):
                tb = KPG * qb + j
                obuf = ob_pool.tile([128, D], BF, name="obuf", tag="ob")
                for hh in range(2):
                    # the last projection runs after the scores pool is done
                    # with attention: alternate with it for a 4-slot po
                    # pipeline (2 rotation slots per pool)
                    use_s = last and (2 * j + hh) % 2 == 1
                    pool = ps_s if use_s else ps_f
                    po = pool.tile([128, 512], FP, name="po",
                                   tag="ps_s" if use_s else "ps_f")
                    for c in range(2):
                        nc.tensor.matmul(
                            po,
                            lhsT=ctxT[c][:, tb * 128:(tb + 1) * 128],
                            rhs=wproj_sb[:, c, hh * 512:(hh + 1) * 512],
                            start=(c == 0),
                            stop=(c == 1),
                        )
                    yield
                    osl = obuf[:, hh * 512:(hh + 1) * 512]
                    if last:
                        # tail: ACT is done with exps; DVE runs the chunked
                        # normalize that gates these blocks
                        nc.scalar.copy(osl, po)
                    elif hh == 0:
                        nc.vector.tensor_copy(out=osl, in_=po)
                    else:
                        nc.scalar.copy(osl, po)
                    yield
                t0 = tb * 128
                if last:
                    nc.sync.dma_start(out_d[t0:t0 + 128, 0:512], obuf[:, 0:512])
                    nc.gpsimd.dma_start(out_d[t0:t0 + 128, 512:D], obuf[:, 512:D])
                else:
                    eng = nc.sync if tb % 2 == 0 else nc.gpsimd
                    eng.dma_start(out_d[t0:t0 + 128, :], obuf)
                yield

        def run(gen):
            for _ in gen:
                pass

        class Chain:
            """A shared stream of filler units pulled at per-phase rates.

            pull_through(idx) exhausts generators 0..idx -- REQUIRED before
            an attention phase that reads their outputs (emission order is
            dependency order in the tile framework: a read emitted before
            its writer sees stale memory)."""

            def __init__(self, gens):
                self.gens = list(gens)
                self.gi = 0
                self.credit = 0.0

            def _step(self):
                while self.gi < len(self.gens):
                    try:
                        next(self.gens[self.gi])
                        return True
                    except StopIteration:
                        self.gi += 1
                return False

            def pull(self, n):
                self.credit += n
                while self.credit >= 1.0 and self._step():
                    self.credit -= 1.0

            def pull_through(self, idx):
                while self.gi <= idx:
                    if not self._step():
                        break

            def drain(self):
                while self._step():
                    pass

        def weave(primary, s1c, rate, proj_gen=None, proj_rate=0.0):
            """Emit attention rounds interleaved with stage1-chain units
            (global, carried across phases) and this phase's proj units."""
            pq = Chain([proj_gen]) if proj_gen is not None else None
            for _ in primary:
                s1c.pull(rate)
                if pq is not None:
                    pq.pull(proj_rate)
            if pq is not None:
                pq.drain()

        def whole():
            if phases == "s1":
                for g in range(NG):
                    run(stage1(g, 0))
                    run(stage1(g, 1))
                return
            run(stage1(0, 0))
            s1c = Chain([
                stage1(0, 1), stage1(1, 0), stage1(1, 1), stage1(2, 0),
                stage1(2, 1), stage1(3, 0), stage1(3, 1),
            ])
            # chain index of s1(qb, half) is 2*qb + half - 1; attention(qb, r)
            # reads qT[r]/kT[r]/vt written by s1(qb, r) from its FIRST round
            # (the query group), so that generator must be exhausted first.
            prefetch(1)
            weave(attention(0, 0), s1c, 2.5)
            s1c.pull_through(0)
            weave(attention(0, 1), s1c, 2.5)
            prefetch(2)
            s1c.pull_through(1)
            weave(attention(1, 0), s1c, 1.0)
            s1c.pull_through(2)
            weave(attention(1, 1), s1c, 1.0, proj(0), 2.0)
            prefetch(3)
            s1c.pull_through(3)
            weave(attention(2, 0), s1c, 0.8)
            s1c.pull_through(4)
            weave(attention(2, 1), s1c, 0.8, proj(1), 1.43)
            s1c.pull_through(5)
            weave(attention(3, 0), s1c, 0.8)
            s1c.pull_through(6)
            weave(attention(3, 1), s1c, 1.0, proj(2), 1.1)
            s1c.drain()
            run(proj(3))
            if dbg:
                for c in range(2):
                    nc.sync.dma_start(qt_dbg[c], qT[c])
                    nc.sync.dma_start(kt_dbg[c], kT[c])
                    nc.sync.dma_start(ctx_dbg[c], ctxT[c])
                nc.sync.dma_start(vt_dbg, vt)

        if loop_n == 1:
            whole()
        else:
            with tc.For_i(0, loop_n, 1):
                whole()

    nc.compile()
    return nc


@functools.lru_cache(maxsize=4)
def _get_program(loop_n=1, phases="all", opts=""):
    return _build_program(loop_n, phases, opts)


def _host_inputs(x, w_qkv, w_proj):
    """Build the 8 per-core input maps from the full problem inputs."""
    from ml_dtypes import bfloat16

    x = np.asarray(x, dtype=np.float32)
    w_qkv = np.asarray(w_qkv, dtype=np.float32)
    w_proj = np.asarray(w_proj, dtype=np.float32)

    # x transposed per batch: (KC, 128, S) bf16
    xt = {
        b: np.ascontiguousarray(x[b].T).reshape(KC, 128, S).astype(bfloat16)
        for b in range(B)
    }

    # RoPE tables, transposed + pair-replicated; sin is sign-folded.
    inv_freq = 1.0 / (ROPE_BASE ** (np.arange(0, Dh, 2, dtype=np.float32) / Dh))
    tpos = np.arange(S, dtype=np.float32)
    freqs = np.outer(tpos, inv_freq)                      # (S, 32)
    emb = np.concatenate([freqs, freqs], axis=-1)         # (S, 64)
    cosT = np.cos(emb).T.astype(np.float32)               # (64, S)
    sinT = np.sin(emb).T.astype(np.float32)
    sinT_f = sinT.copy()
    sinT_f[:32] *= -1.0                                   # fold rotate_half sign
    cos_full = np.ascontiguousarray(np.tile(cosT, (2, 1))).astype(bfloat16)
    sin_full = np.ascontiguousarray(np.tile(sinT_f, (2, 1))).astype(bfloat16)

    r = np.arange(128)
    tri = (r[None, :] >= r[:, None]).astype(bfloat16)     # tri[r, c] = c >= r

    wq = w_qkv[:, 0:D]
    wk = w_qkv[:, D:2 * D]
    wv = w_qkv[:, 2 * D:3 * D]

    in_maps = []
    for c in range(N_CORES):
        b, hq = divmod(c, HPC)
        # wqk half-major: [half, KC, 128, q|k|v x 128]
        wqk_c = np.stack([
            np.concatenate(
                [w[:, np.r_[(4 * hq + 2 * half) * 64:(4 * hq + 2 * half + 2) * 64]]
                 for w in (wq, wk, wv)], axis=1
            ).reshape(KC, 128, 384)
            for half in range(2)
        ])  # (2, KC, 128, 384)
        in_maps.append({
            "xt": xt[b],
            "wqk": np.ascontiguousarray(wqk_c).astype(bfloat16),
            "wproj": np.ascontiguousarray(
                w_proj[4 * hq * 64:(4 * hq + 4) * 64, :]
            ).reshape(2, 128, D).astype(bfloat16),
            "cost": cos_full,
            "sint": sin_full,
            "tri": tri,
            "onesc": np.ones((128, NKB, 65 * HPC), dtype=bfloat16),
            "onesr": np.ones((1, 64), dtype=np.float32),
            "ident": np.eye(128, dtype=bfloat16),
        })
    return in_maps


_last_results = None


def kernel(x, w_qkv, w_proj):
    global _last_results
    from concourse.bass_utils import run_bass_kernel_spmd

    nc = _get_program()
    in_maps = _host_inputs(x, w_qkv, w_proj)
    trace = bool(int(os.environ.get("KERNEL_TRACE", "0")))
    kwargs = {}
    if trace:
        kwargs["trace"] = True
        kwargs["trace_cores"] = list(range(N_CORES))
    res = run_bass_kernel_spmd(nc, in_maps, core_ids=list(range(N_CORES)), **kwargs)
    _last_results = res
    acc = np.zeros((B, S, D), dtype=np.float32)
    for c, r in enumerate(res.results):
        acc[c // HPC] += r["out"].astype(np.float32)
    return acc


# revision 64
# speedup vs baseline: 1.1308x; 1.1308x over previous
"""Trainium2 Bass kernel for MultiHeadAttentionRoPE.

Problem (hardcoded): B=2, S=2048, D=1024, H=16 heads, Dh=64, fp32 I/O.
    qkv = x @ w_qkv ; q,k -> RoPE ; causal attention ; out = ctx @ w_proj

Sharding: core = (batch, head-quad). Each of the 8 cores handles one of
the 2 batches and 4 of the 16 heads: it reads its batch's x (bf16,
transposed on host), its 4-head slice of w_qkv/w_proj, computes causal
attention for those heads and a *partial* projection output [S, D]; the
host sums the 4 partials per batch (the gather step of row-parallel TP).

All matmul operands are bf16 (fp32 PSUM accumulation); rel-err vs the
fp32 reference is ~6e-3 (tolerance 2e-2).

v2: 512-token pipeline. Everything is tiled in 512-token groups (vs 1024
in v1) and emission is software-pipelined so the in-order PE queue always
has work while ACT chews the exp stream (ACT-exp is within ~25% of the PE
roofline, so attention rounds alone starve the PE):

  s1(0,0) | a(0,0)+s1(0,1) | a(0,1)+s1(1,0) | a(1,0)+s1(1,1) |
  a(1,1)+s1(2,0)+proj(0) | a(2,0)+s1(2,1) | a(2,1)+s1(3,0)+proj(1) |
  a(3,0)+s1(3,1) | a(3,1)+proj(2) | proj(3)

Each attention call's rounds are interleaved with "filler" units (stage1
psq chunks, RoPE, projection halves) pulled from the next pipeline stage.

PSUM (8 banks of 2KB): scores 2x[128,2,512]f32 (both heads of a round in
one tile so each round runs ONE exp over 1024 cols -- the ACT per-inst
PSUM-access overhead is ~143ns and ACT is within ~25% of PE), pv
2x[65,512]f32, filler pool 2x[128,512]f32 for stage1 psq / v-transpose
scratch / projection column-halves.

Per-core layout (as v1):
  - x fed transposed (d on partitions); head pairs packed per 128-partition
    tile (h_even rows 0:64, h_odd 64:128) in qT/kT/ctxT.
  - RoPE: rotate-half via partition-swapping SBUF->SBUF DMAs on a bf16
    staging tile, sin sign-folded on host; combine on DVE (bf16 2x mode).
  - scores transposed (keys on partitions, queries free); exp on ACT;
    denominator from a ones-column appended per head in the vt blocks
    (ones written by a strided Pool memset, not a DRAM load).
  - causal: key-blocks above the diagonal skipped; diagonal 128-blocks get
    a triangular 0/1 mask multiply (DVE, bf16 2x).
  - normalize: den row psum->sbuf (DVE), fast approx reciprocal (DVE),
    partition-broadcast (Pool), applied in the psum->sbuf ctx downcast.
  - projection per 128-token block in 512-col halves; partials staged in
    sbuf and stored per-half on the SP queue.
"""

import functools
import os
import sys

import numpy as np

sys.path.insert(0, "/opt/trn_rl_repo")

# ---- problem constants (must match reference.py) ----
B = 2
S = 2048
D = 1024
H = 16
Dh = 64
N_CORES = 8
HPC = 4                     # heads per core
KC = D // 128               # contraction chunks = 8
TCH = 512                   # token chunk (group / query block)
NG = S // TCH               # 4 groups
KPG = TCH // 128            # key blocks per group = 4
NKB = S // 128              # 16 key blocks
ROPE_BASE = 10000.0
SCALE = 1.0 / 8.0           # 1/sqrt(Dh)


def _build_program(loop_n=1, phases="all", opts=""):
    import concourse.bass as bass
    opts = set(opts.split(",")) if opts else set()
    import concourse.mybir as mybir
    import concourse.tile as tile
    from concourse import bacc
    from contextlib import ExitStack

    FP = mybir.dt.float32
    BF = mybir.dt.bfloat16
    FPR = mybir.dt.float32r
    EXP = mybir.ActivationFunctionType.Exp
    R = lambda ap: ap.bitcast(FPR)

    nc = bacc.Bacc("TRN2", target_bir_lowering=False, debug=False)

    xt_d = nc.dram_tensor("xt", [KC, 128, S], BF, kind="ExternalInput").ap()
    # wqk half-major: [half, kc, 128, q|k|v x 128] so stage1(0,0) only waits
    # on the half-0 blocks at startup
    wqk_d = nc.dram_tensor("wqk", [2, KC, 128, 3 * 128], BF, kind="ExternalInput").ap()
    wproj_d = nc.dram_tensor("wproj", [2, 128, D], BF, kind="ExternalInput").ap()
    cos_d = nc.dram_tensor("cost", [128, S], BF, kind="ExternalInput").ap()
    sin_d = nc.dram_tensor("sint", [128, S], BF, kind="ExternalInput").ap()
    tri_d = nc.dram_tensor("tri", [128, 128], BF, kind="ExternalInput").ap()
    ones_d = nc.dram_tensor("onesc", [128, NKB, 65 * HPC], BF, kind="ExternalInput").ap()
    onesr_d = nc.dram_tensor("onesr", [1, 64], FP, kind="ExternalInput").ap()
    ident_d = nc.dram_tensor("ident", [128, 128], BF, kind="ExternalInput").ap()
    out_d = nc.dram_tensor("out", [S, D], BF, kind="ExternalOutput").ap()
    dbg = phases == "dbg"
    if dbg:
        qt_dbg = nc.dram_tensor("qt_dbg", [2, 128, S], BF, kind="ExternalOutput").ap()
        kt_dbg = nc.dram_tensor("kt_dbg", [2, 128, S], BF, kind="ExternalOutput").ap()
        vt_dbg = nc.dram_tensor("vt_dbg", [128, NKB, 65 * HPC], BF, kind="ExternalOutput").ap()
        ctx_dbg = nc.dram_tensor("ctx_dbg", [2, 128, S], BF, kind="ExternalOutput").ap()

    with tile.TileContext(nc) as tc, ExitStack() as ctx:
        consts = ctx.enter_context(tc.tile_pool(name="consts", bufs=1))
        store = ctx.enter_context(tc.tile_pool(name="store", bufs=1))
        xt_pool = ctx.enter_context(tc.tile_pool(name="xt_pool", bufs=3))
        rt_pool = ctx.enter_context(tc.tile_pool(name="rt_pool", bufs=2))
        p_pool = ctx.enter_context(tc.tile_pool(name="p_pool", bufs=6))
        nrm_pool = ctx.enter_context(tc.tile_pool(name="nrm_pool", bufs=2))
        ob_pool = ctx.enter_context(tc.tile_pool(name="ob_pool", bufs=4))
        # PSUM: scores 3 banks, pv 3 banks, filler 2 banks (psq / v-transpose
        # scratch / proj halves).
        ps_s = ctx.enter_context(tc.tile_pool(name="ps_s", bufs=2, space="PSUM"))
        ps_v = ctx.enter_context(tc.tile_pool(name="ps_v", bufs=2, space="PSUM"))
        ps_f = ctx.enter_context(tc.tile_pool(name="ps_f", bufs=2, space="PSUM"))

        # ---- constants ----
        wqk_sb = consts.tile([128, 2, KC, 3 * 128], BF, name="wqk_sb")
        ident_sb = consts.tile([128, 128], BF, name="ident_sb")
        wproj_sb = consts.tile([128, 2, D], BF, name="wproj_sb")
        cos_sb = consts.tile([128, S], BF, name="cos_sb")
        sin_sb = consts.tile([128, S], BF, name="sin_sb")
        tri_sb = consts.tile([128, 128], BF, name="tri_sb")
        onesr_sb = consts.tile([1, 64], FP, name="onesr_sb")

        # ---- persistent per-core storage ----
        # qT/kT chunk c holds heads (2c, 2c+1): rows [h dh0..63 | h' dh0..63]
        qT = {c: store.tile([128, S], BF, name=f"qT_{c}", tag=f"qT_{c}") for c in range(2)}
        kT = {c: store.tile([128, S], BF, name=f"kT_{c}", tag=f"kT_{c}") for c in range(2)}
        ctxT = {c: store.tile([128, S], BF, name=f"ctxT_{c}", tag=f"ctxT_{c}") for c in range(2)}
        # vt: per key block kb, cols [v_h0 |1| v_h1 |1| v_h2 |1| v_h3 |1]
        vt = store.tile([128, NKB, 65 * HPC], BF, name="vt", tag="vt")

        def late_consts():
            # Queue discipline: the SP/Pool queues carry the startup-critical
            # rotate-half swap DMAs at ~6-10us; anything bulky emitted before
            # those (in-order queues!) delays the first scores. Only small /
            # immediately-needed consts load here; the bulk loads are emitted
            # AFTER stage1(0,0) (see whole()).
            with tc.tile_wait_until(0.003):
                # group-0 RoPE table slices (RoPE at ~7-10us needs cols 0:TCH)
                nc.sync.dma_start(cos_sb[:, 0:TCH], cos_d[:, 0:TCH])
                nc.scalar.dma_start(sin_sb[:, 0:TCH], sin_d[:, 0:TCH])
                nc.sync.dma_start(R(onesr_sb), R(onesr_d))
            # ident (v transposes at ~9us) right behind the startup chunks;
            # NOTE tile_wait fractions are of a ~960us scheduler estimate
            # (0.001 ~ 1us), so gates here are near-absolute microseconds
            nc.gpsimd.dma_start(ident_sb, ident_d)
            nc.gpsimd.dma_start(tri_sb, tri_d)
            # vt ones init MUST be emitted before stage1(0,0)'s v interleave
            # copy (emission order is write order); ACT's HWDGE is quiet at
            # startup and doesn't carry the critical rotate-half swaps
            nc.scalar.dma_start(vt[:, 0:KPG], ones_d[:, 0:KPG])
            nc.scalar.dma_start(vt[:, KPG:NKB], ones_d[:, KPG:NKB])
            with tc.tile_wait_until(0.010):
                for kc in range(KC):
                    eng = nc.sync if kc % 2 == 0 else nc.gpsimd
                    eng.dma_start(wqk_sb[:, 1, kc, :], wqk_d[1, kc])

        def bulk_consts():
            """Emitted after stage1(0,0): the startup-critical swap DMAs are
            already queued ahead. Ordered by first-use time. vt ones init is
            a full-tile write (col 64 of each 65-block stays 1; the v
            interleave copies overwrite the rest) -- a strided column DMA
            racing the interleave writes corrupted adjacent values on HW in
            v1. Split so the first PV (key block 0, ~13us) isn't gated on
            the whole 1MB (vt ones itself is emitted in late_consts -- it
            must precede the v interleave writes)."""
            nc.sync.dma_start(cos_sb[:, TCH:S], cos_d[:, TCH:S])
            with tc.tile_wait_until(0.012):
                nc.gpsimd.dma_start(sin_sb[:, TCH:S], sin_d[:, TCH:S])
                for i in range(2):
                    nc.gpsimd.dma_start(wproj_sb[:, i, :], wproj_d[i])

        xtiles = {}

        def prefetch(g):
            """Issue group g's x loads ~a phase before stage1(g) runs."""
            xtile = xt_pool.tile([128, KC, TCH], BF, name="xtile", tag="xt")
            xtiles[g] = xtile
            tsl = slice(g * TCH, (g + 1) * TCH)
            for kc in range(KC):
                nc.gpsimd.dma_start(xtile[:, kc, :], xt_d[kc, :, tsl])

        def stage1(g, half):
            """QKV^T projection + RoPE + v natural layout for token group g
            (512 tokens), head pair `half`. Yields between units so the
            driver can weave it into attention rounds."""
            tsl = slice(g * TCH, (g + 1) * TCH)
            if half == 0 and g == 0:
                xtile = xt_pool.tile([128, KC, TCH], BF, name="xtile", tag="xt")
                xtiles[g] = xtile
                for kc in range(KC):
                    # startup-critical: spread x + half-0 wqk across the
                    # SP/Pool/ACT queues, each kc's pair on two queues
                    q3 = (nc.sync, nc.gpsimd, nc.scalar)
                    q3[kc % 3].dma_start(xtile[:, kc, :], xt_d[kc, :, tsl])
                    q3[(kc + 1) % 3].dma_start(wqk_sb[:, 0, kc, :], wqk_d[0, kc])
                late_consts()
            xtile = xtiles[g]
            rtile = rt_pool.tile([128, 2, TCH], BF, name="rtile", tag="rt")
            qs = rt_pool.tile([128, 2, TCH], BF, name="qs", tag="qs")
            # i: 0 = q, 1 = k, 2 = v; blk indexes the half's [q|k|v] blocks.
            # q first: its RoPE chain gates the next phase's first scores.
            for i, blk in enumerate((0, 1, 2)):
                psq = ps_f.tile([128, TCH], FP, name="psq", tag="ps_f")
                for kc in range(KC):
                    nc.tensor.matmul(
                        psq,
                        lhsT=wqk_sb[:, half, kc, blk * 128:(blk + 1) * 128],
                        rhs=xtile[:, kc, :],
                        start=(kc == 0),
                        stop=(kc == KC - 1),
                    )
                    if kc == 3:
                        yield
                yield
                if i < 2:
                    # stage q/k in bf16 for the rotate-half partition swap.
                    # g==0: ACT (idle at startup; DVE runs the RoPE chains
                    # that gate the first scores). g>0: DVE (ACT runs exps).
                    if g == 0:
                        nc.scalar.copy(rtile[:, i, :], psq)
                    else:
                        nc.vector.tensor_copy(out=rtile[:, i, :], in_=psq)
                    # issue this operand's rotate-half partition swaps
                    # immediately (k's swaps overlap q's projection),
                    # spread across the SP/Pool/ACT queues
                    qeng = (
                        (nc.sync, nc.gpsimd, nc.scalar, nc.sync) if g == 0
                        else (nc.sync, nc.gpsimd, nc.sync, nc.gpsimd)
                    )
                    for j, (d0, s0) in enumerate(
                        ((0, 32), (32, 0), (64, 96), (96, 64))
                    ):
                        qeng[j].dma_start(
                            qs[d0:d0 + 32, i, :], rtile[s0:s0 + 32, i, :]
                        )
                    if i == 1:
                        yield
                        # q first: the first score matmul needs all of qT but
                        # only the first key block of kT
                        order = ((0, qT[half]), (1, kT[half]))
                        for j, dest in order:
                            t1 = rt_pool.tile([128, TCH], BF, name="t1", tag="t1")
                            nc.vector.tensor_mul(out=t1, in0=qs[:, j, :], in1=sin_sb[:, tsl])
                            nc.vector.tensor_mul(out=dest[:, tsl], in0=rtile[:, j, :], in1=cos_sb[:, tsl])
                            nc.vector.tensor_add(out=dest[:, tsl], in0=dest[:, tsl], in1=t1)
                            yield
                else:
                    # v -> natural layout via PE transposes
                    vts = rt_pool.tile([128, TCH], BF, name="vts", tag="vts")
                    if g == 0:
                        nc.scalar.copy(vts, psq)
                    else:
                        nc.vector.tensor_copy(out=vts, in_=psq)
                    yield
                    pv4 = ps_f.tile([128, KPG, 128], BF, name="pv4", tag="ps_f")
                    for sc in range(KPG):
                        nc.tensor.transpose(
                            pv4[:, sc, :], vts[:, sc * 128:(sc + 1) * 128], ident_sb
                        )
                    yield
                    # interleave into vt: head pair (2*half, 2*half+1)
                    v2 = vt[:, g * KPG:(g + 1) * KPG, :].rearrange(
                        "p k (a c) -> p k a c", c=65
                    )[:, :, 2 * half:2 * half + 2, 0:64]
                    s2 = pv4.rearrange("p k (a c) -> p k a c", c=64)
                    nc.vector.tensor_copy(out=v2, in_=s2)
                    yield

        def attention(qb, r):
            """Causal attention for 512-token query block qb, head pair r.
            Yields once per key-block round (the weave points)."""
            q0 = qb * TCH
            nkc = KPG * qb + KPG
            hs = (2 * r, 2 * r + 1)

            def score_exp(kc):
                """Both heads' scores into one 2-bank psum tile -> ONE exp
                (halves the ACT per-instruction PSUM-access overhead)."""
                off = max(0, (kc - KPG * qb) * 128)
                c = r
                ps = ps_s.tile([128, 2, TCH], FP, name="ps", tag="ps_s")
                for hi in range(2):
                    hb = hi * 64
                    nc.tensor.matmul(
                        ps[:, hi, off:TCH],
                        lhsT=kT[c][hb:hb + 64, kc * 128:(kc + 1) * 128],
                        rhs=qT[c][hb:hb + 64, q0 + off:q0 + TCH],
                        start=True,
                        stop=True,
                    )
                p = p_pool.tile([128, 2, TCH], BF, name="p", tag="p")
                nc.scalar.activation(p[:, :, off:TCH], ps[:, :, off:TCH], EXP, scale=SCALE)
                if kc >= KPG * qb:  # diagonal band: triangular mask
                    # DVE: tiny (bf16 2x) and on the exp->PV critical path;
                    # Pool would queue it behind DMA descriptor generation
                    eng = nc.gpsimd if "poolmask" in opts else nc.vector
                    for hi in range(2):
                        eng.tensor_mul(
                            out=p[:, hi, off:off + 128],
                            in0=p[:, hi, off:off + 128],
                            in1=tri_sb,
                        )
                return p

            pv = {h: ps_v.tile([65, TCH], FP, name=f"ppv{h}", tag="ps_v") for h in hs}

            def pv_mm(kc, p):
                off = max(0, (kc - KPG * qb) * 128)
                for hi, h in enumerate(hs):
                    nc.tensor.matmul(
                        pv[h][:, off:TCH],
                        lhsT=vt[:, kc, 65 * h:65 * h + 65],
                        rhs=p[:, hi, off:TCH],
                        start=(kc == 0),
                        stop=(kc == nkc - 1),
                    )

            pk = {}
            for kc in range(nkc):
                pk[kc] = score_exp(kc)
                if kc >= 2:
                    pv_mm(kc - 2, pk.pop(kc - 2))
                yield
            for kc in (nkc - 2, nkc - 1):
                if kc >= 0:
                    pv_mm(kc, pk.pop(kc))
            # normalize: ctx rows 0..63 per head, denominator row 64.
            def normalize(h, c0, c1, den_act=False):
                hb = (h % 2) * 64
                c = h // 2
                n = c1 - c0
                den = nrm_pool.tile([1, TCH], FP, name="den", tag="den")
                dn = den[:, 0:n]
                if den_act:
                    # tail: stage this head's den on ACT so both heads'
                    # chains overlap (ACT is done with exps by then)
                    nc.scalar.copy(dn, pv[h][64:65, c0:c1])
                else:
                    nc.vector.tensor_copy(out=R(dn), in_=pv[h][64:65, c0:c1])
                rcp = nrm_pool.tile([1, TCH], FP, name="rcp", tag="rcp")
                rp = rcp[:, 0:n]
                nc.vector.reciprocal_approx_fast(out=rp, in_=dn)
                rcb = nrm_pool.tile([64, TCH], FP, name="rcb", tag="rcb")
                rb = rcb[:, 0:n]
                nc.gpsimd.partition_broadcast(rb, rp, channels=64)
                nc.vector.tensor_mul(
                    out=ctxT[c][hb:hb + 64, q0 + c0:q0 + c1],
                    in0=pv[h][0:64, c0:c1], in1=rb,
                )

            if qb == NG - 1:
                # 256-col chunks so proj(3)'s first token blocks start after
                # the first chunk; h1's den staged on ACT to overlap chains
                for ch in range(2):
                    for hi, h in enumerate(hs):
                        normalize(h, ch * 256, (ch + 1) * 256, den_act=(hi == 1))
                    yield
            else:
                for h in hs:
                    normalize(h, 0, TCH)
                    yield

        def proj(qb):
            """Projection for query block qb (4 token blocks of 128), in
            512-col halves so each occupies one filler PSUM bank. Drains go
            to DVE while attention exps overlap; the final block's drain is
            split DVE/ACT with half-stores on two queues to shorten the
            kernel tail."""
            last = qb == NG - 1
            for j in range(KPG):
                tb = KPG * qb + j
                obuf = ob_pool.tile([128, D], BF, name="obuf", tag="ob")
                for hh in range(2):
                    # the last projection runs after the scores pool is done
                    # with attention: alternate with it for a 4-slot po
                    # pipeline (2 rotation slots per pool)
                    use_s = last and (2 * j + hh) % 2 == 1
                    pool = ps_s if use_s else ps_f
                    po = pool.tile([128, 512], FP, name="po",
                                   tag="ps_s" if use_s else "ps_f")
                    for c in range(2):
                        nc.tensor.matmul(
                            po,
                            lhsT=ctxT[c][:, tb * 128:(tb + 1) * 128],
                            rhs=wproj_sb[:, c, hh * 512:(hh + 1) * 512],
                            start=(c == 0),
                            stop=(c == 1),
                        )
                    yield
                    osl = obuf[:, hh * 512:(hh + 1) * 512]
                    if last and j == KPG - 1:
                        # final block: parallel DVE/ACT drain halves for
                        # minimum latency before the final stores
                        nc.vector.tensor_copy(out=osl[:, 0:256], in_=po[:, 0:256])
                        nc.scalar.copy(osl[:, 256:512], po[:, 256:512])
                    elif last:
                        # ACT: DVE runs the chunked normalize gating these
                        nc.scalar.copy(osl, po)
                    else:
                        # DVE: keeps the exp-critical ACT queue clear
                        nc.vector.tensor_copy(out=osl, in_=po)
                    yield
                t0 = tb * 128
                if last and j == KPG - 1:
                    # piecewise stores, each waiting only its drain piece;
                    # the final piece rides the lower-latency SP HWDGE
                    for pc, eng in enumerate(
                        (nc.sync, nc.gpsimd, nc.gpsimd, nc.sync)
                    ):
                        c0 = pc * 256
                        eng.dma_start(
                            out_d[t0:t0 + 128, c0:c0 + 256], obuf[:, c0:c0 + 256]
                        )
                elif last:
                    nc.sync.dma_start(out_d[t0:t0 + 128, 0:512], obuf[:, 0:512])
                    nc.gpsimd.dma_start(out_d[t0:t0 + 128, 512:D], obuf[:, 512:D])
                else:
                    eng = nc.sync if tb % 2 == 0 else nc.gpsimd
                    eng.dma_start(out_d[t0:t0 + 128, :], obuf)
                yield

        def run(gen):
            for _ in gen:
                pass

        class Chain:
            """A shared stream of filler units pulled at per-phase rates.

            pull_through(idx) exhausts generators 0..idx -- REQUIRED before
            an attention phase that reads their outputs (emission order is
            dependency order in the tile framework: a read emitted before
            its writer sees stale memory)."""

            def __init__(self, gens):
                self.gens = list(gens)
                self.gi = 0
                self.credit = 0.0

            def _step(self):
                while self.gi < len(self.gens):
                    try:
                        next(self.gens[self.gi])
                        return True
                    except StopIteration:
                        self.gi += 1
                return False

            def pull(self, n):
                self.credit += n
                while self.credit >= 1.0 and self._step():
                    self.credit -= 1.0

            def pull_through(self, idx):
                while self.gi <= idx:
                    if not self._step():
                        break

            def drain(self):
                while self._step():
                    pass

        def weave(primary, s1c, rate, proj_gen=None, proj_rate=0.0):
            """Emit attention rounds interleaved with stage1-chain units
            (global, carried across phases) and this phase's proj units."""
            pq = Chain([proj_gen]) if proj_gen is not None else None
            for _ in primary:
                s1c.pull(rate)
                if pq is not None:
                    pq.pull(proj_rate)
            if pq is not None:
                pq.drain()

        def whole():
            if phases == "s1":
                for g in range(NG):
                    run(stage1(g, 0))
                    run(stage1(g, 1))
                return
            run(stage1(0, 0))
            s1c = Chain([
                stage1(0, 1), stage1(1, 0), stage1(1, 1), stage1(2, 0),
                stage1(2, 1), stage1(3, 0), stage1(3, 1),
            ])
            # chain index of s1(qb, half) is 2*qb + half - 1; attention(qb, r)
            # reads qT[r]/kT[r]/vt written by s1(qb, r) from its FIRST round
            # (the query group), so that generator must be exhausted first.
            bulk_consts()
            prefetch(1)
            weave(attention(0, 0), s1c, 2.5)
            s1c.pull_through(0)
            weave(attention(0, 1), s1c, 2.5)
            prefetch(2)
            s1c.pull_through(1)
            weave(attention(1, 0), s1c, 1.0)
            s1c.pull_through(2)
            weave(attention(1, 1), s1c, 1.0, proj(0), 2.0)
            prefetch(3)
            s1c.pull_through(3)
            weave(attention(2, 0), s1c, 1.0)
            s1c.pull_through(4)
            weave(attention(2, 1), s1c, 1.0, proj(1), 1.43)
            s1c.pull_through(5)
            weave(attention(3, 0), s1c, 0.7)
            s1c.pull_through(6)
            weave(attention(3, 1), s1c, 1.0, proj(2), 1.1)
            s1c.drain()
            run(proj(3))
            if dbg:
                for c in range(2):
                    nc.sync.dma_start(qt_dbg[c], qT[c])
                    nc.sync.dma_start(kt_dbg[c], kT[c])
                    nc.sync.dma_start(ctx_dbg[c], ctxT[c])
                nc.sync.dma_start(vt_dbg, vt)

        if loop_n == 1:
            whole()
        else:
            with tc.For_i(0, loop_n, 1):
                whole()

    nc.compile()
    return nc


@functools.lru_cache(maxsize=4)
def _get_program(loop_n=1, phases="all", opts=""):
    return _build_program(loop_n, phases, opts)


def _host_inputs(x, w_qkv, w_proj):
    """Build the 8 per-core input maps from the full problem inputs."""
    from ml_dtypes import bfloat16

    x = np.asarray(x, dtype=np.float32)
    w_qkv = np.asarray(w_qkv, dtype=np.float32)
    w_proj = np.asarray(w_proj, dtype=np.float32)

    # x transposed per batch: (KC, 128, S) bf16
    xt = {
        b: np.ascontiguousarray(x[b].T).reshape(KC, 128, S).astype(bfloat16)
        for b in range(B)
    }

    # RoPE tables, transposed + pair-replicated; sin is sign-folded.
    inv_freq = 1.0 / (ROPE_BASE ** (np.arange(0, Dh, 2, dtype=np.float32) / Dh))
    tpos = np.arange(S, dtype=np.float32)
    freqs = np.outer(tpos, inv_freq)                      # (S, 32)
    emb = np.concatenate([freqs, freqs], axis=-1)         # (S, 64)
    cosT = np.cos(emb).T.astype(np.float32)               # (64, S)
    sinT = np.sin(emb).T.astype(np.float32)
    sinT_f = sinT.copy()
    sinT_f[:32] *= -1.0                                   # fold rotate_half sign
    cos_full = np.ascontiguousarray(np.tile(cosT, (2, 1))).astype(bfloat16)
    sin_full = np.ascontiguousarray(np.tile(sinT_f, (2, 1))).astype(bfloat16)

    r = np.arange(128)
    tri = (r[None, :] >= r[:, None]).astype(bfloat16)     # tri[r, c] = c >= r

    wq = w_qkv[:, 0:D]
    wk = w_qkv[:, D:2 * D]
    wv = w_qkv[:, 2 * D:3 * D]

    in_maps = []
    for c in range(N_CORES):
        b, hq = divmod(c, HPC)
        # wqk half-major: [half, KC, 128, q|k|v x 128]
        wqk_c = np.stack([
            np.concatenate(
                [w[:, np.r_[(4 * hq + 2 * half) * 64:(4 * hq + 2 * half + 2) * 64]]
                 for w in (wq, wk, wv)], axis=1
            ).reshape(KC, 128, 384)
            for half in range(2)
        ])  # (2, KC, 128, 384)
        in_maps.append({
            "xt": xt[b],
            "wqk": np.ascontiguousarray(wqk_c).astype(bfloat16),
            "wproj": np.ascontiguousarray(
                w_proj[4 * hq * 64:(4 * hq + 4) * 64, :]
            ).reshape(2, 128, D).astype(bfloat16),
            "cost": cos_full,
            "sint": sin_full,
            "tri": tri,
            "onesc": np.ones((128, NKB, 65 * HPC), dtype=bfloat16),
            "onesr": np.ones((1, 64), dtype=np.float32),
            "ident": np.eye(128, dtype=bfloat16),
        })
    return in_maps


_last_results = None


def kernel(x, w_qkv, w_proj):
    global _last_results
    from concourse.bass_utils import run_bass_kernel_spmd

    nc = _get_program()
    in_maps = _host_inputs(x, w_qkv, w_proj)
    trace = bool(int(os.environ.get("KERNEL_TRACE", "0")))
    kwargs = {}
    if trace:
        kwargs["trace"] = True
        kwargs["trace_cores"] = list(range(N_CORES))
    res = run_bass_kernel_spmd(nc, in_maps, core_ids=list(range(N_CORES)), **kwargs)
    _last_results = res
    acc = np.zeros((B, S, D), dtype=np.float32)
    for c, r in enumerate(res.results):
        acc[c // HPC] += r["out"].astype(np.float32)
    return acc


# revision 67
# speedup vs baseline: 1.1396x; 1.0078x over previous
"""Trainium2 Bass kernel for MultiHeadAttentionRoPE.

Problem (hardcoded): B=2, S=2048, D=1024, H=16 heads, Dh=64, fp32 I/O.
    qkv = x @ w_qkv ; q,k -> RoPE ; causal attention ; out = ctx @ w_proj

Sharding: core = (batch, head-quad). Each of the 8 cores handles one of
the 2 batches and 4 of the 16 heads: it reads its batch's x (bf16,
transposed on host), its 4-head slice of w_qkv/w_proj, computes causal
attention for those heads and a *partial* projection output [S, D]; the
host sums the 4 partials per batch (the gather step of row-parallel TP).

All matmul operands are bf16 (fp32 PSUM accumulation); rel-err vs the
fp32 reference is ~6e-3 (tolerance 2e-2).

v2: 512-token pipeline. Everything is tiled in 512-token groups (vs 1024
in v1) and emission is software-pipelined so the in-order PE queue always
has work while ACT chews the exp stream (ACT-exp is within ~25% of the PE
roofline, so attention rounds alone starve the PE):

  s1(0,0) | a(0,0)+s1(0,1) | a(0,1)+s1(1,0) | a(1,0)+s1(1,1) |
  a(1,1)+s1(2,0)+proj(0) | a(2,0)+s1(2,1) | a(2,1)+s1(3,0)+proj(1) |
  a(3,0)+s1(3,1) | a(3,1)+proj(2) | proj(3)

Each attention call's rounds are interleaved with "filler" units (stage1
psq chunks, RoPE, projection halves) pulled from the next pipeline stage.

PSUM (8 banks of 2KB): scores 2x[128,2,512]f32 (both heads of a round in
one tile so each round runs ONE exp over 1024 cols -- the ACT per-inst
PSUM-access overhead is ~143ns and ACT is within ~25% of PE), pv
2x[65,512]f32, filler pool 2x[128,512]f32 for stage1 psq / v-transpose
scratch / projection column-halves.

Per-core layout (as v1):
  - x fed transposed (d on partitions); head pairs packed per 128-partition
    tile (h_even rows 0:64, h_odd 64:128) in qT/kT/ctxT.
  - RoPE: rotate-half via partition-swapping SBUF->SBUF DMAs on a bf16
    staging tile, sin sign-folded on host; combine on DVE (bf16 2x mode).
  - scores transposed (keys on partitions, queries free); exp on ACT;
    denominator from a ones-column appended per head in the vt blocks
    (ones written by a strided Pool memset, not a DRAM load).
  - causal: key-blocks above the diagonal skipped; diagonal 128-blocks get
    a triangular 0/1 mask multiply (DVE, bf16 2x).
  - normalize: den row psum->sbuf (DVE), fast approx reciprocal (DVE),
    partition-broadcast (Pool), applied in the psum->sbuf ctx downcast.
  - projection per 128-token block in 512-col halves; partials staged in
    sbuf and stored per-half on the SP queue.
"""

import functools
import os
import sys

import numpy as np

sys.path.insert(0, "/opt/trn_rl_repo")

# ---- problem constants (must match reference.py) ----
B = 2
S = 2048
D = 1024
H = 16
Dh = 64
N_CORES = 8
HPC = 4                     # heads per core
KC = D // 128               # contraction chunks = 8
TCH = 512                   # token chunk (group / query block)
NG = S // TCH               # 4 groups
KPG = TCH // 128            # key blocks per group = 4
NKB = S // 128              # 16 key blocks
ROPE_BASE = 10000.0
SCALE = 1.0 / 8.0           # 1/sqrt(Dh)


def _build_program(loop_n=1, phases="all", opts=""):
    import concourse.bass as bass
    opts = set(opts.split(",")) if opts else set()
    import concourse.mybir as mybir
    import concourse.tile as tile
    from concourse import bacc
    from contextlib import ExitStack

    FP = mybir.dt.float32
    BF = mybir.dt.bfloat16
    FPR = mybir.dt.float32r
    EXP = mybir.ActivationFunctionType.Exp
    R = lambda ap: ap.bitcast(FPR)

    nc = bacc.Bacc("TRN2", target_bir_lowering=False, debug=False)

    xt_d = nc.dram_tensor("xt", [KC, 128, S], BF, kind="ExternalInput").ap()
    # wqk half-major: [half, kc, 128, q|k|v x 128] so stage1(0,0) only waits
    # on the half-0 blocks at startup
    wqk_d = nc.dram_tensor("wqk", [2, KC, 128, 3 * 128], BF, kind="ExternalInput").ap()
    wproj_d = nc.dram_tensor("wproj", [2, 128, D], BF, kind="ExternalInput").ap()
    cos_d = nc.dram_tensor("cost", [128, S], BF, kind="ExternalInput").ap()
    sin_d = nc.dram_tensor("sint", [128, S], BF, kind="ExternalInput").ap()
    tri_d = nc.dram_tensor("tri", [128, 128], BF, kind="ExternalInput").ap()
    ones_d = nc.dram_tensor("onesc", [128, NKB, 65 * HPC], BF, kind="ExternalInput").ap()
    onesr_d = nc.dram_tensor("onesr", [1, 64], FP, kind="ExternalInput").ap()
    ident_d = nc.dram_tensor("ident", [128, 128], BF, kind="ExternalInput").ap()
    out_d = nc.dram_tensor("out", [S, D], BF, kind="ExternalOutput").ap()
    dbg = phases == "dbg"
    if dbg:
        qt_dbg = nc.dram_tensor("qt_dbg", [2, 128, S], BF, kind="ExternalOutput").ap()
        kt_dbg = nc.dram_tensor("kt_dbg", [2, 128, S], BF, kind="ExternalOutput").ap()
        vt_dbg = nc.dram_tensor("vt_dbg", [128, NKB, 65 * HPC], BF, kind="ExternalOutput").ap()
        ctx_dbg = nc.dram_tensor("ctx_dbg", [2, 128, S], BF, kind="ExternalOutput").ap()

    with tile.TileContext(nc) as tc, ExitStack() as ctx:
        consts = ctx.enter_context(tc.tile_pool(name="consts", bufs=1))
        store = ctx.enter_context(tc.tile_pool(name="store", bufs=1))
        xt_pool = ctx.enter_context(tc.tile_pool(name="xt_pool", bufs=3))
        rt_pool = ctx.enter_context(tc.tile_pool(name="rt_pool", bufs=2))
        p_pool = ctx.enter_context(tc.tile_pool(name="p_pool", bufs=6))
        nrm_pool = ctx.enter_context(tc.tile_pool(name="nrm_pool", bufs=2))
        ob_pool = ctx.enter_context(tc.tile_pool(name="ob_pool", bufs=4))
        # PSUM: scores 3 banks, pv 3 banks, filler 2 banks (psq / v-transpose
        # scratch / proj halves).
        ps_s = ctx.enter_context(tc.tile_pool(name="ps_s", bufs=2, space="PSUM"))
        ps_v = ctx.enter_context(tc.tile_pool(name="ps_v", bufs=2, space="PSUM"))
        ps_f = ctx.enter_context(tc.tile_pool(name="ps_f", bufs=2, space="PSUM"))

        # ---- constants ----
        wqk_sb = consts.tile([128, 2, KC, 3 * 128], BF, name="wqk_sb")
        ident_sb = consts.tile([128, 128], BF, name="ident_sb")
        wproj_sb = consts.tile([128, 2, D], BF, name="wproj_sb")
        cos_sb = consts.tile([128, S], BF, name="cos_sb")
        sin_sb = consts.tile([128, S], BF, name="sin_sb")
        tri_sb = consts.tile([128, 128], BF, name="tri_sb")
        onesr_sb = consts.tile([1, 64], FP, name="onesr_sb")

        # ---- persistent per-core storage ----
        # qT/kT chunk c holds heads (2c, 2c+1): rows [h dh0..63 | h' dh0..63]
        qT = {c: store.tile([128, S], BF, name=f"qT_{c}", tag=f"qT_{c}") for c in range(2)}
        kT = {c: store.tile([128, S], BF, name=f"kT_{c}", tag=f"kT_{c}") for c in range(2)}
        ctxT = {c: store.tile([128, S], BF, name=f"ctxT_{c}", tag=f"ctxT_{c}") for c in range(2)}
        # vt: per key block kb, cols [v_h0 |1| v_h1 |1| v_h2 |1| v_h3 |1]
        vt = store.tile([128, NKB, 65 * HPC], BF, name="vt", tag="vt")

        def late_consts():
            # Queue discipline: the SP/Pool queues carry the startup-critical
            # rotate-half swap DMAs at ~6-10us; anything bulky emitted before
            # those (in-order queues!) delays the first scores. Only small /
            # immediately-needed consts load here; the bulk loads are emitted
            # AFTER stage1(0,0) (see whole()).
            with tc.tile_wait_until(0.003):
                # group-0 RoPE table slices (RoPE at ~7-10us needs cols 0:TCH)
                nc.gpsimd.dma_start(cos_sb[:, 0:TCH], cos_d[:, 0:TCH])
                nc.scalar.dma_start(sin_sb[:, 0:TCH], sin_d[:, 0:TCH])
                nc.gpsimd.dma_start(R(onesr_sb), R(onesr_d))
            # ident (v transposes at ~9us) right behind the startup chunks;
            # NOTE tile_wait fractions are of a ~960us scheduler estimate
            # (0.001 ~ 1us), so gates here are near-absolute microseconds
            nc.gpsimd.dma_start(ident_sb, ident_d)
            nc.gpsimd.dma_start(tri_sb, tri_d)
            # vt ones init MUST be emitted before stage1(0,0)'s v interleave
            # copy (emission order is write order); ACT's HWDGE is quiet at
            # startup and doesn't carry the critical rotate-half swaps
            nc.scalar.dma_start(vt[:, 0:KPG], ones_d[:, 0:KPG])
            nc.scalar.dma_start(vt[:, KPG:NKB], ones_d[:, KPG:NKB])
            with tc.tile_wait_until(0.010):
                for kc in range(KC):
                    eng = nc.sync if kc % 2 == 0 else nc.gpsimd
                    eng.dma_start(wqk_sb[:, 1, kc, :], wqk_d[1, kc])

        def bulk_consts():
            """Emitted after stage1(0,0): the startup-critical swap DMAs are
            already queued ahead. Ordered by first-use time. vt ones init is
            a full-tile write (col 64 of each 65-block stays 1; the v
            interleave copies overwrite the rest) -- a strided column DMA
            racing the interleave writes corrupted adjacent values on HW in
            v1. Split so the first PV (key block 0, ~13us) isn't gated on
            the whole 1MB (vt ones itself is emitted in late_consts -- it
            must precede the v interleave writes)."""
            nc.sync.dma_start(cos_sb[:, TCH:S], cos_d[:, TCH:S])
            with tc.tile_wait_until(0.012):
                nc.gpsimd.dma_start(sin_sb[:, TCH:S], sin_d[:, TCH:S])
                for i in range(2):
                    nc.gpsimd.dma_start(wproj_sb[:, i, :], wproj_d[i])

        xtiles = {}

        def prefetch(g):
            """Issue group g's x loads ~a phase before stage1(g) runs."""
            xtile = xt_pool.tile([128, KC, TCH], BF, name="xtile", tag="xt")
            xtiles[g] = xtile
            tsl = slice(g * TCH, (g + 1) * TCH)
            for kc in range(KC):
                nc.gpsimd.dma_start(xtile[:, kc, :], xt_d[kc, :, tsl])

        def stage1(g, half):
            """QKV^T projection + RoPE + v natural layout for token group g
            (512 tokens), head pair `half`. Yields between units so the
            driver can weave it into attention rounds."""
            tsl = slice(g * TCH, (g + 1) * TCH)
            if half == 0 and g == 0:
                xtile = xt_pool.tile([128, KC, TCH], BF, name="xtile", tag="xt")
                xtiles[g] = xtile
                for kc in range(KC):
                    # startup-critical: x + half-0 wqk on SP/Pool only --
                    # every ACT-queue DMA costs ~650ns of ACT sequencer time
                    # and delays the rtile drains that gate RoPE
                    q2 = (nc.sync, nc.gpsimd)
                    q2[kc % 2].dma_start(xtile[:, kc, :], xt_d[kc, :, tsl])
                    q2[(kc + 1) % 2].dma_start(wqk_sb[:, 0, kc, :], wqk_d[0, kc])
                late_consts()
            xtile = xtiles[g]
            rtile = rt_pool.tile([128, 2, TCH], BF, name="rtile", tag="rt")
            qs = rt_pool.tile([128, 2, TCH], BF, name="qs", tag="qs")
            # i: 0 = q, 1 = k, 2 = v; blk indexes the half's [q|k|v] blocks.
            # q first: its RoPE chain gates the next phase's first scores.
            for i, blk in enumerate((0, 1, 2)):
                psq = ps_f.tile([128, TCH], FP, name="psq", tag="ps_f")
                for kc in range(KC):
                    nc.tensor.matmul(
                        psq,
                        lhsT=wqk_sb[:, half, kc, blk * 128:(blk + 1) * 128],
                        rhs=xtile[:, kc, :],
                        start=(kc == 0),
                        stop=(kc == KC - 1),
                    )
                    if kc == 3:
                        yield
                yield
                if i < 2:
                    # stage q/k in bf16 for the rotate-half partition swap.
                    # g==0: ACT (idle at startup; DVE runs the RoPE chains
                    # that gate the first scores). g>0: DVE (ACT runs exps).
                    if g == 0:
                        nc.scalar.copy(rtile[:, i, :], psq)
                    else:
                        nc.vector.tensor_copy(out=rtile[:, i, :], in_=psq)
                    # issue this operand's rotate-half partition swaps
                    # immediately (k's swaps overlap q's projection),
                    # spread across the SP/Pool/ACT queues
                    qeng = (
                        (nc.sync, nc.gpsimd, nc.scalar, nc.sync) if g == 0
                        else (nc.sync, nc.gpsimd, nc.sync, nc.gpsimd)
                    )
                    for j, (d0, s0) in enumerate(
                        ((0, 32), (32, 0), (64, 96), (96, 64))
                    ):
                        qeng[j].dma_start(
                            qs[d0:d0 + 32, i, :], rtile[s0:s0 + 32, i, :]
                        )
                    if i == 1:
                        yield
                        # q first: the first score matmul needs all of qT but
                        # only the first key block of kT
                        order = ((0, qT[half]), (1, kT[half]))
                        for j, dest in order:
                            t1 = rt_pool.tile([128, TCH], BF, name="t1", tag="t1")
                            nc.vector.tensor_mul(out=t1, in0=qs[:, j, :], in1=sin_sb[:, tsl])
                            nc.vector.tensor_mul(out=dest[:, tsl], in0=rtile[:, j, :], in1=cos_sb[:, tsl])
                            nc.vector.tensor_add(out=dest[:, tsl], in0=dest[:, tsl], in1=t1)
                            yield
                else:
                    # v -> natural layout via PE transposes
                    vts = rt_pool.tile([128, TCH], BF, name="vts", tag="vts")
                    if g == 0:
                        nc.scalar.copy(vts, psq)
                    else:
                        nc.vector.tensor_copy(out=vts, in_=psq)
                    yield
                    pv4 = ps_f.tile([128, KPG, 128], BF, name="pv4", tag="ps_f")
                    for sc in range(KPG):
                        nc.tensor.transpose(
                            pv4[:, sc, :], vts[:, sc * 128:(sc + 1) * 128], ident_sb
                        )
                    yield
                    # interleave into vt: head pair (2*half, 2*half+1)
                    v2 = vt[:, g * KPG:(g + 1) * KPG, :].rearrange(
                        "p k (a c) -> p k a c", c=65
                    )[:, :, 2 * half:2 * half + 2, 0:64]
                    s2 = pv4.rearrange("p k (a c) -> p k a c", c=64)
                    nc.vector.tensor_copy(out=v2, in_=s2)
                    yield

        def attention(qb, r):
            """Causal attention for 512-token query block qb, head pair r.
            Yields once per key-block round (the weave points)."""
            q0 = qb * TCH
            nkc = KPG * qb + KPG
            hs = (2 * r, 2 * r + 1)

            def score_exp(kc):
                """Both heads' scores into one 2-bank psum tile -> ONE exp
                (halves the ACT per-instruction PSUM-access overhead)."""
                off = max(0, (kc - KPG * qb) * 128)
                c = r
                ps = ps_s.tile([128, 2, TCH], FP, name="ps", tag="ps_s")
                for hi in range(2):
                    hb = hi * 64
                    nc.tensor.matmul(
                        ps[:, hi, off:TCH],
                        lhsT=kT[c][hb:hb + 64, kc * 128:(kc + 1) * 128],
                        rhs=qT[c][hb:hb + 64, q0 + off:q0 + TCH],
                        start=True,
                        stop=True,
                    )
                p = p_pool.tile([128, 2, TCH], BF, name="p", tag="p")
                nc.scalar.activation(p[:, :, off:TCH], ps[:, :, off:TCH], EXP, scale=SCALE)
                if kc >= KPG * qb:  # diagonal band: triangular mask
                    # DVE: tiny (bf16 2x) and on the exp->PV critical path;
                    # Pool would queue it behind DMA descriptor generation
                    eng = nc.gpsimd if "poolmask" in opts else nc.vector
                    for hi in range(2):
                        eng.tensor_mul(
                            out=p[:, hi, off:off + 128],
                            in0=p[:, hi, off:off + 128],
                            in1=tri_sb,
                        )
                return p

            pv = {h: ps_v.tile([65, TCH], FP, name=f"ppv{h}", tag="ps_v") for h in hs}

            def pv_mm(kc, p):
                off = max(0, (kc - KPG * qb) * 128)
                for hi, h in enumerate(hs):
                    nc.tensor.matmul(
                        pv[h][:, off:TCH],
                        lhsT=vt[:, kc, 65 * h:65 * h + 65],
                        rhs=p[:, hi, off:TCH],
                        start=(kc == 0),
                        stop=(kc == nkc - 1),
                    )

            pk = {}
            for kc in range(nkc):
                pk[kc] = score_exp(kc)
                if kc >= 2:
                    pv_mm(kc - 2, pk.pop(kc - 2))
                yield
            for kc in (nkc - 2, nkc - 1):
                if kc >= 0:
                    pv_mm(kc, pk.pop(kc))
            # normalize: ctx rows 0..63 per head, denominator row 64.
            def normalize(h, c0, c1, den_act=False):
                hb = (h % 2) * 64
                c = h // 2
                n = c1 - c0
                den = nrm_pool.tile([1, TCH], FP, name="den", tag="den")
                dn = den[:, 0:n]
                if den_act:
                    # tail: stage this head's den on ACT so both heads'
                    # chains overlap (ACT is done with exps by then)
                    nc.scalar.copy(dn, pv[h][64:65, c0:c1])
                else:
                    nc.vector.tensor_copy(out=R(dn), in_=pv[h][64:65, c0:c1])
                rcp = nrm_pool.tile([1, TCH], FP, name="rcp", tag="rcp")
                rp = rcp[:, 0:n]
                nc.vector.reciprocal_approx_fast(out=rp, in_=dn)
                rcb = nrm_pool.tile([64, TCH], FP, name="rcb", tag="rcb")
                rb = rcb[:, 0:n]
                nc.gpsimd.partition_broadcast(rb, rp, channels=64)
                nc.vector.tensor_mul(
                    out=ctxT[c][hb:hb + 64, q0 + c0:q0 + c1],
                    in0=pv[h][0:64, c0:c1], in1=rb,
                )

            if qb == NG - 1:
                # 256-col chunks so proj(3)'s first token blocks start after
                # the first chunk; h1's den staged on ACT to overlap chains
                for ch in range(2):
                    for hi, h in enumerate(hs):
                        normalize(h, ch * 256, (ch + 1) * 256, den_act=(hi == 1))
                    yield
            else:
                for h in hs:
                    normalize(h, 0, TCH)
                    yield

        def proj(qb):
            """Projection for query block qb (4 token blocks of 128), in
            512-col halves so each occupies one filler PSUM bank. Drains go
            to DVE while attention exps overlap; the final block's drain is
            split DVE/ACT with half-stores on two queues to shorten the
            kernel tail."""
            last = qb == NG - 1
            for j in range(KPG):
                tb = KPG * qb + j
                obuf = ob_pool.tile([128, D], BF, name="obuf", tag="ob")
                for hh in range(2):
                    # the last projection runs after the scores pool is done
                    # with attention: alternate with it for a 4-slot po
                    # pipeline (2 rotation slots per pool)
                    use_s = last and (2 * j + hh) % 2 == 1
                    pool = ps_s if use_s else ps_f
                    po = pool.tile([128, 512], FP, name="po",
                                   tag="ps_s" if use_s else "ps_f")
                    for c in range(2):
                        nc.tensor.matmul(
                            po,
                            lhsT=ctxT[c][:, tb * 128:(tb + 1) * 128],
                            rhs=wproj_sb[:, c, hh * 512:(hh + 1) * 512],
                            start=(c == 0),
                            stop=(c == 1),
                        )
                    yield
                    osl = obuf[:, hh * 512:(hh + 1) * 512]
                    if last and j >= KPG - 2:
                        # late blocks: normalize is done -- parallel DVE/ACT
                        # drain halves for minimum latency into the stores
                        nc.vector.tensor_copy(out=osl[:, 0:256], in_=po[:, 0:256])
                        nc.scalar.copy(osl[:, 256:512], po[:, 256:512])
                    elif last:
                        # ACT (exp-free by now): DVE still runs the chunked
                        # normalize gating the later blocks
                        nc.scalar.copy(osl, po)
                    else:
                        # DVE: keeps the exp-critical ACT queue clear
                        nc.vector.tensor_copy(out=osl, in_=po)
                    yield
                t0 = tb * 128
                if last and j == KPG - 1:
                    # piecewise stores, each waiting only its drain piece;
                    # the final piece rides the lower-latency SP HWDGE
                    for pc, eng in enumerate(
                        (nc.sync, nc.gpsimd, nc.gpsimd, nc.sync)
                    ):
                        c0 = pc * 256
                        eng.dma_start(
                            out_d[t0:t0 + 128, c0:c0 + 256], obuf[:, c0:c0 + 256]
                        )
                elif last:
                    nc.sync.dma_start(out_d[t0:t0 + 128, 0:512], obuf[:, 0:512])
                    nc.gpsimd.dma_start(out_d[t0:t0 + 128, 512:D], obuf[:, 512:D])
                else:
                    eng = nc.sync if tb % 2 == 0 else nc.gpsimd
                    eng.dma_start(out_d[t0:t0 + 128, :], obuf)
                yield

        def run(gen):
            for _ in gen:
                pass

        class Chain:
            """A shared stream of filler units pulled at per-phase rates.

            pull_through(idx) exhausts generators 0..idx -- REQUIRED before
            an attention phase that reads their outputs (emission order is
            dependency order in the tile framework: a read emitted before
            its writer sees stale memory)."""

            def __init__(self, gens):
                self.gens = list(gens)
                self.gi = 0
                self.credit = 0.0

            def _step(self):
                while self.gi < len(self.gens):
                    try:
                        next(self.gens[self.gi])
                        return True
                    except StopIteration:
                        self.gi += 1
                return False

            def pull(self, n):
                self.credit += n
                while self.credit >= 1.0 and self._step():
                    self.credit -= 1.0

            def pull_through(self, idx):
                while self.gi <= idx:
                    if not self._step():
                        break

            def drain(self):
                while self._step():
                    pass

        def weave(primary, s1c, rate, proj_gen=None, proj_rate=0.0):
            """Emit attention rounds interleaved with stage1-chain units
            (global, carried across phases) and this phase's proj units."""
            pq = Chain([proj_gen]) if proj_gen is not None else None
            for _ in primary:
                s1c.pull(rate)
                if pq is not None:
                    pq.pull(proj_rate)
            if pq is not None:
                pq.drain()

        def whole():
            if phases == "s1":
                for g in range(NG):
                    run(stage1(g, 0))
                    run(stage1(g, 1))
                return
            run(stage1(0, 0))
            s1c = Chain([
                stage1(0, 1), stage1(1, 0), stage1(1, 1), stage1(2, 0),
                stage1(2, 1), stage1(3, 0), stage1(3, 1),
            ])
            # chain index of s1(qb, half) is 2*qb + half - 1; attention(qb, r)
            # reads qT[r]/kT[r]/vt written by s1(qb, r) from its FIRST round
            # (the query group), so that generator must be exhausted first.
            bulk_consts()
            prefetch(1)
            weave(attention(0, 0), s1c, 2.5)
            s1c.pull_through(0)
            weave(attention(0, 1), s1c, 2.5)
            prefetch(2)
            s1c.pull_through(1)
            weave(attention(1, 0), s1c, 1.0)
            s1c.pull_through(2)
            weave(attention(1, 1), s1c, 1.0, proj(0), 2.0)
            prefetch(3)
            s1c.pull_through(3)
            weave(attention(2, 0), s1c, 1.0)
            s1c.pull_through(4)
            weave(attention(2, 1), s1c, 1.0, proj(1), 1.43)
            s1c.pull_through(5)
            weave(attention(3, 0), s1c, 0.7)
            s1c.pull_through(6)
            weave(attention(3, 1), s1c, 1.0, proj(2), 1.1)
            s1c.drain()
            run(proj(3))
            if dbg:
                for c in range(2):
                    nc.sync.dma_start(qt_dbg[c], qT[c])
                    nc.sync.dma_start(kt_dbg[c], kT[c])
                    nc.sync.dma_start(ctx_dbg[c], ctxT[c])
                nc.sync.dma_start(vt_dbg, vt)

        if loop_n == 1:
            whole()
        else:
            with tc.For_i(0, loop_n, 1):
                whole()

    nc.compile()
    return nc


@functools.lru_cache(maxsize=4)
def _get_program(loop_n=1, phases="all", opts=""):
    return _build_program(loop_n, phases, opts)


def _host_inputs(x, w_qkv, w_proj):
    """Build the 8 per-core input maps from the full problem inputs."""
    from ml_dtypes import bfloat16

    x = np.asarray(x, dtype=np.float32)
    w_qkv = np.asarray(w_qkv, dtype=np.float32)
    w_proj = np.asarray(w_proj, dtype=np.float32)

    # x transposed per batch: (KC, 128, S) bf16
    xt = {
        b: np.ascontiguousarray(x[b].T).reshape(KC, 128, S).astype(bfloat16)
        for b in range(B)
    }

    # RoPE tables, transposed + pair-replicated; sin is sign-folded.
    inv_freq = 1.0 / (ROPE_BASE ** (np.arange(0, Dh, 2, dtype=np.float32) / Dh))
    tpos = np.arange(S, dtype=np.float32)
    freqs = np.outer(tpos, inv_freq)                      # (S, 32)
    emb = np.concatenate([freqs, freqs], axis=-1)         # (S, 64)
    cosT = np.cos(emb).T.astype(np.float32)               # (64, S)
    sinT = np.sin(emb).T.astype(np.float32)
    sinT_f = sinT.copy()
    sinT_f[:32] *= -1.0                                   # fold rotate_half sign
    cos_full = np.ascontiguousarray(np.tile(cosT, (2, 1))).astype(bfloat16)
    sin_full = np.ascontiguousarray(np.tile(sinT_f, (2, 1))).astype(bfloat16)

    r = np.arange(128)
    tri = (r[None, :] >= r[:, None]).astype(bfloat16)     # tri[r, c] = c >= r

    wq = w_qkv[:, 0:D]
    wk = w_qkv[:, D:2 * D]
    wv = w_qkv[:, 2 * D:3 * D]

    in_maps = []
    for c in range(N_CORES):
        b, hq = divmod(c, HPC)
        # wqk half-major: [half, KC, 128, q|k|v x 128]
        wqk_c = np.stack([
            np.concatenate(
                [w[:, np.r_[(4 * hq + 2 * half) * 64:(4 * hq + 2 * half + 2) * 64]]
                 for w in (wq, wk, wv)], axis=1
            ).reshape(KC, 128, 384)
            for half in range(2)
        ])  # (2, KC, 128, 384)
        in_maps.append({
            "xt": xt[b],
            "wqk": np.ascontiguousarray(wqk_c).astype(bfloat16),
            "wproj": np.ascontiguousarray(
                w_proj[4 * hq * 64:(4 * hq + 4) * 64, :]
            ).reshape(2, 128, D).astype(bfloat16),
            "cost": cos_full,
            "sint": sin_full,
            "tri": tri,
            "onesc": np.ones((128, NKB, 65 * HPC), dtype=bfloat16),
            "onesr": np.ones((1, 64), dtype=np.float32),
            "ident": np.eye(128, dtype=bfloat16),
        })
    return in_maps


_last_results = None


def kernel(x, w_qkv, w_proj):
    global _last_results
    from concourse.bass_utils import run_bass_kernel_spmd

    nc = _get_program()
    in_maps = _host_inputs(x, w_qkv, w_proj)
    trace = bool(int(os.environ.get("KERNEL_TRACE", "0")))
    kwargs = {}
    if trace:
        kwargs["trace"] = True
        kwargs["trace_cores"] = list(range(N_CORES))
    res = run_bass_kernel_spmd(nc, in_maps, core_ids=list(range(N_CORES)), **kwargs)
    _last_results = res
    acc = np.zeros((B, S, D), dtype=np.float32)
    for c, r in enumerate(res.results):
        acc[c // HPC] += r["out"].astype(np.float32)
    return acc


# revision 76
# speedup vs baseline: 1.1655x; 1.0228x over previous
"""Trainium2 Bass kernel for MultiHeadAttentionRoPE.

Problem (hardcoded): B=2, S=2048, D=1024, H=16 heads, Dh=64, fp32 I/O.
    qkv = x @ w_qkv ; q,k -> RoPE ; causal attention ; out = ctx @ w_proj

Sharding: core = (batch, head-quad). Each of the 8 cores handles one of
the 2 batches and 4 of the 16 heads: it reads its batch's x (bf16,
transposed on host), its 4-head slice of w_qkv/w_proj, computes causal
attention for those heads and a *partial* projection output [S, D]; the
host sums the 4 partials per batch (the gather step of row-parallel TP).

All matmul operands are bf16 (fp32 PSUM accumulation); rel-err vs the
fp32 reference is ~6e-3 (tolerance 2e-2).

v2: 512-token pipeline. Everything is tiled in 512-token groups (vs 1024
in v1) and emission is software-pipelined so the in-order PE queue always
has work while ACT chews the exp stream (ACT-exp is within ~25% of the PE
roofline, so attention rounds alone starve the PE):

  s1(0,0) | a(0,0)+s1(0,1) | a(0,1)+s1(1,0) | a(1,0)+s1(1,1) |
  a(1,1)+s1(2,0)+proj(0) | a(2,0)+s1(2,1) | a(2,1)+s1(3,0)+proj(1) |
  a(3,0)+s1(3,1) | a(3,1)+proj(2) | proj(3)

Each attention call's rounds are interleaved with "filler" units (stage1
psq chunks, RoPE, projection halves) pulled from the next pipeline stage.

PSUM (8 banks of 2KB): scores 2x[128,2,512]f32 (both heads of a round in
one tile so each round runs ONE exp over 1024 cols -- the ACT per-inst
PSUM-access overhead is ~143ns and ACT is within ~25% of PE), pv
2x[65,512]f32, filler pool 2x[128,512]f32 for stage1 psq / v-transpose
scratch / projection column-halves.

Per-core layout (as v1):
  - x fed transposed (d on partitions); head pairs packed per 128-partition
    tile (h_even rows 0:64, h_odd 64:128) in qT/kT/ctxT.
  - RoPE: rotate-half via partition-swapping SBUF->SBUF DMAs on a bf16
    staging tile, sin sign-folded on host; combine on DVE (bf16 2x mode).
  - scores transposed (keys on partitions, queries free); exp on ACT;
    denominator from a ones-column appended per head in the vt blocks
    (ones written by a strided Pool memset, not a DRAM load).
  - causal: key-blocks above the diagonal skipped; diagonal 128-blocks get
    a triangular 0/1 mask multiply (DVE, bf16 2x).
  - normalize: den row psum->sbuf (DVE), fast approx reciprocal (DVE),
    partition-broadcast (Pool), applied in the psum->sbuf ctx downcast.
  - projection per 128-token block in 512-col halves; partials staged in
    sbuf and stored per-half on the SP queue.
"""

import functools
import os
import sys

import numpy as np

sys.path.insert(0, "/opt/trn_rl_repo")

# ---- problem constants (must match reference.py) ----
B = 2
S = 2048
D = 1024
H = 16
Dh = 64
N_CORES = 8
HPC = 4                     # heads per core
KC = D // 128               # contraction chunks = 8
TCH = 512                   # token chunk (group / query block)
NG = S // TCH               # 4 groups
KPG = TCH // 128            # key blocks per group = 4
NKB = S // 128              # 16 key blocks
ROPE_BASE = 10000.0
SCALE = 1.0 / 8.0           # 1/sqrt(Dh)


def _build_program(loop_n=1, phases="all", opts=""):
    import concourse.bass as bass
    opts = set(opts.split(",")) if opts else set()
    import concourse.mybir as mybir
    import concourse.tile as tile
    from concourse import bacc
    from contextlib import ExitStack

    FP = mybir.dt.float32
    BF = mybir.dt.bfloat16
    FPR = mybir.dt.float32r
    EXP = mybir.ActivationFunctionType.Exp
    R = lambda ap: ap.bitcast(FPR)

    nc = bacc.Bacc("TRN2", target_bir_lowering=False, debug=False)

    xt_d = nc.dram_tensor("xt", [KC, 128, S], BF, kind="ExternalInput").ap()
    # wqk half-major: [half, kc, 128, q|k|v x 128] so stage1(0,0) only waits
    # on the half-0 blocks at startup
    wqk_d = nc.dram_tensor("wqk", [2, KC, 128, 3 * 128], BF, kind="ExternalInput").ap()
    wproj_d = nc.dram_tensor("wproj", [2, 128, D], BF, kind="ExternalInput").ap()
    cos_d = nc.dram_tensor("cost", [128, S], BF, kind="ExternalInput").ap()
    sin_d = nc.dram_tensor("sint", [128, S], BF, kind="ExternalInput").ap()
    tri_d = nc.dram_tensor("tri", [128, 128], BF, kind="ExternalInput").ap()
    ones_d = nc.dram_tensor("onesc", [128, NKB, 65 * HPC], BF, kind="ExternalInput").ap()
    onesr_d = nc.dram_tensor("onesr", [1, 64], FP, kind="ExternalInput").ap()
    ident_d = nc.dram_tensor("ident", [128, 128], BF, kind="ExternalInput").ap()
    out_d = nc.dram_tensor("out", [S, D], BF, kind="ExternalOutput").ap()
    dbg = phases == "dbg"
    if dbg:
        qt_dbg = nc.dram_tensor("qt_dbg", [2, 128, S], BF, kind="ExternalOutput").ap()
        kt_dbg = nc.dram_tensor("kt_dbg", [2, 128, S], BF, kind="ExternalOutput").ap()
        vt_dbg = nc.dram_tensor("vt_dbg", [128, NKB, 65 * HPC], BF, kind="ExternalOutput").ap()
        ctx_dbg = nc.dram_tensor("ctx_dbg", [2, 128, S], BF, kind="ExternalOutput").ap()

    with tile.TileContext(nc) as tc, ExitStack() as ctx:
        consts = ctx.enter_context(tc.tile_pool(name="consts", bufs=1))
        store = ctx.enter_context(tc.tile_pool(name="store", bufs=1))
        xt_pool = ctx.enter_context(tc.tile_pool(name="xt_pool", bufs=3))
        rt_pool = ctx.enter_context(tc.tile_pool(name="rt_pool", bufs=2))
        p_pool = ctx.enter_context(tc.tile_pool(name="p_pool", bufs=6))
        nrm_pool = ctx.enter_context(tc.tile_pool(name="nrm_pool", bufs=2))
        ob_pool = ctx.enter_context(tc.tile_pool(name="ob_pool", bufs=4))
        # PSUM: scores 3 banks, pv 3 banks, filler 2 banks (psq / v-transpose
        # scratch / proj halves).
        ps_s = ctx.enter_context(tc.tile_pool(name="ps_s", bufs=2, space="PSUM"))
        ps_v = ctx.enter_context(tc.tile_pool(name="ps_v", bufs=2, space="PSUM"))
        ps_f = ctx.enter_context(tc.tile_pool(name="ps_f", bufs=2, space="PSUM"))

        # ---- constants ----
        wqk_sb = consts.tile([128, 2, KC, 3 * 128], BF, name="wqk_sb")
        ident_sb = consts.tile([128, 128], BF, name="ident_sb")
        wproj_sb = consts.tile([128, 2, D], BF, name="wproj_sb")
        cos_sb = consts.tile([128, S], BF, name="cos_sb")
        sin_sb = consts.tile([128, S], BF, name="sin_sb")
        tri_sb = consts.tile([128, 128], BF, name="tri_sb")
        onesr_sb = consts.tile([1, 64], FP, name="onesr_sb")

        # ---- persistent per-core storage ----
        # qT/kT chunk c holds heads (2c, 2c+1): rows [h dh0..63 | h' dh0..63]
        qT = {c: store.tile([128, S], BF, name=f"qT_{c}", tag=f"qT_{c}") for c in range(2)}
        kT = {c: store.tile([128, S], BF, name=f"kT_{c}", tag=f"kT_{c}") for c in range(2)}
        ctxT = {c: store.tile([128, S], BF, name=f"ctxT_{c}", tag=f"ctxT_{c}") for c in range(2)}
        # vt: per key block kb, cols [v_h0 |1| v_h1 |1| v_h2 |1| v_h3 |1]
        vt = store.tile([128, NKB, 65 * HPC], BF, name="vt", tag="vt")

        def late_consts():
            # Queue discipline: the SP/Pool queues carry the startup-critical
            # rotate-half swap DMAs at ~6-10us; anything bulky emitted before
            # those (in-order queues!) delays the first scores. Only small /
            # immediately-needed consts load here; the bulk loads are emitted
            # AFTER stage1(0,0) (see whole()).
            with tc.tile_wait_until(0.003):
                # group-0 RoPE table slices (RoPE at ~7-10us needs cols 0:TCH)
                nc.gpsimd.dma_start(cos_sb[:, 0:TCH], cos_d[:, 0:TCH])
                nc.scalar.dma_start(sin_sb[:, 0:TCH], sin_d[:, 0:TCH])
                nc.gpsimd.dma_start(R(onesr_sb), R(onesr_d))
            # ident (v transposes at ~9us) right behind the startup chunks;
            # NOTE tile_wait fractions are of a ~960us scheduler estimate
            # (0.001 ~ 1us), so gates here are near-absolute microseconds
            nc.gpsimd.dma_start(ident_sb, ident_d)
            nc.gpsimd.dma_start(tri_sb, tri_d)
            # vt ones init MUST be emitted before stage1(0,0)'s v interleave
            # copy (emission order is write order); ACT's HWDGE is quiet at
            # startup and doesn't carry the critical rotate-half swaps
            nc.scalar.dma_start(vt[:, 0:KPG], ones_d[:, 0:KPG])
            nc.scalar.dma_start(vt[:, KPG:NKB], ones_d[:, KPG:NKB])
            with tc.tile_wait_until(0.010):
                for kc in range(KC):
                    eng = nc.sync if kc % 2 == 0 else nc.gpsimd
                    eng.dma_start(wqk_sb[:, 1, kc, :], wqk_d[1, kc])

        def bulk_consts():
            """Emitted after stage1(0,0): the startup-critical swap DMAs are
            already queued ahead. Ordered by first-use time. vt ones init is
            a full-tile write (col 64 of each 65-block stays 1; the v
            interleave copies overwrite the rest) -- a strided column DMA
            racing the interleave writes corrupted adjacent values on HW in
            v1. Split so the first PV (key block 0, ~13us) isn't gated on
            the whole 1MB (vt ones itself is emitted in late_consts -- it
            must precede the v interleave writes)."""
            nc.sync.dma_start(cos_sb[:, TCH:S], cos_d[:, TCH:S])
            with tc.tile_wait_until(0.012):
                nc.gpsimd.dma_start(sin_sb[:, TCH:S], sin_d[:, TCH:S])
                for i in range(2):
                    nc.gpsimd.dma_start(wproj_sb[:, i, :], wproj_d[i])

        xtiles = {}

        def prefetch(g):
            """Issue group g's x loads ~a phase before stage1(g) runs."""
            xtile = xt_pool.tile([128, KC, TCH], BF, name="xtile", tag="xt")
            xtiles[g] = xtile
            tsl = slice(g * TCH, (g + 1) * TCH)
            for kc in range(KC):
                nc.gpsimd.dma_start(xtile[:, kc, :], xt_d[kc, :, tsl])

        def stage1(g, half):
            """QKV^T projection + RoPE + v natural layout for token group g
            (512 tokens), head pair `half`. Yields between units so the
            driver can weave it into attention rounds."""
            tsl = slice(g * TCH, (g + 1) * TCH)
            if half == 0 and g == 0:
                xtile = xt_pool.tile([128, KC, TCH], BF, name="xtile", tag="xt")
                xtiles[g] = xtile
                for kc in range(KC):
                    # startup-critical: x + half-0 wqk on SP/Pool only --
                    # every ACT-queue DMA costs ~650ns of ACT sequencer time
                    # and delays the rtile drains that gate RoPE
                    q2 = (nc.sync, nc.gpsimd)
                    q2[kc % 2].dma_start(xtile[:, kc, :], xt_d[kc, :, tsl])
                    q2[(kc + 1) % 2].dma_start(wqk_sb[:, 0, kc, :], wqk_d[0, kc])
                late_consts()
            xtile = xtiles[g]
            rtile = rt_pool.tile([128, 2, TCH], BF, name="rtile", tag="rt")
            qs = rt_pool.tile([128, 2, TCH], BF, name="qs", tag="qs")
            # i: 0 = q, 1 = k, 2 = v; blk indexes the half's [q|k|v] blocks.
            # q first: its RoPE chain gates the next phase's first scores.
            for i, blk in enumerate((0, 1, 2)):
                psq = ps_f.tile([128, TCH], FP, name="psq", tag="ps_f")
                for kc in range(KC):
                    nc.tensor.matmul(
                        psq,
                        lhsT=wqk_sb[:, half, kc, blk * 128:(blk + 1) * 128],
                        rhs=xtile[:, kc, :],
                        start=(kc == 0),
                        stop=(kc == KC - 1),
                    )
                    if kc == 3:
                        yield
                yield
                if i < 2:
                    # stage q/k in bf16 for the rotate-half partition swap.
                    # g==0: ACT (idle at startup; DVE runs the RoPE chains
                    # that gate the first scores). g>0: DVE (ACT runs exps).
                    if g == 0:
                        nc.scalar.copy(rtile[:, i, :], psq)
                    else:
                        nc.vector.tensor_copy(out=rtile[:, i, :], in_=psq)
                    # issue this operand's rotate-half partition swaps
                    # immediately (k's swaps overlap q's projection),
                    # spread across the SP/Pool/ACT queues
                    qeng = (
                        (nc.sync, nc.gpsimd, nc.scalar, nc.sync) if g == 0
                        else (nc.sync, nc.gpsimd, nc.sync, nc.gpsimd)
                    )
                    for j, (d0, s0) in enumerate(
                        ((0, 32), (32, 0), (64, 96), (96, 64))
                    ):
                        qeng[j].dma_start(
                            qs[d0:d0 + 32, i, :], rtile[s0:s0 + 32, i, :]
                        )
                    if i == 1:
                        yield
                        # q first: the first score matmul needs all of qT but
                        # only the first key block of kT
                        order = ((0, qT[half]), (1, kT[half]))
                        for j, dest in order:
                            t1 = rt_pool.tile([128, TCH], BF, name="t1", tag="t1")
                            nc.vector.tensor_mul(out=t1, in0=qs[:, j, :], in1=sin_sb[:, tsl])
                            nc.vector.tensor_mul(out=dest[:, tsl], in0=rtile[:, j, :], in1=cos_sb[:, tsl])
                            nc.vector.tensor_add(out=dest[:, tsl], in0=dest[:, tsl], in1=t1)
                            yield
                else:
                    # v -> natural layout via PE transposes
                    vts = rt_pool.tile([128, TCH], BF, name="vts", tag="vts")
                    if g == 0:
                        nc.scalar.copy(vts, psq)
                    else:
                        nc.vector.tensor_copy(out=vts, in_=psq)
                    yield
                    pv4 = ps_f.tile([128, KPG, 128], BF, name="pv4", tag="ps_f")
                    for sc in range(KPG):
                        nc.tensor.transpose(
                            pv4[:, sc, :], vts[:, sc * 128:(sc + 1) * 128], ident_sb
                        )
                    yield
                    # interleave into vt: head pair (2*half, 2*half+1)
                    v2 = vt[:, g * KPG:(g + 1) * KPG, :].rearrange(
                        "p k (a c) -> p k a c", c=65
                    )[:, :, 2 * half:2 * half + 2, 0:64]
                    s2 = pv4.rearrange("p k (a c) -> p k a c", c=64)
                    nc.vector.tensor_copy(out=v2, in_=s2)
                    yield

        def attention(qb, r):
            """Causal attention for 512-token query block qb, head pair r.
            Yields once per key-block round (the weave points)."""
            q0 = qb * TCH
            nkc = KPG * qb + KPG
            hs = (2 * r, 2 * r + 1)

            def score_exp(kc):
                """Both heads' scores into one 2-bank psum tile -> ONE exp
                (halves the ACT per-instruction PSUM-access overhead)."""
                off = max(0, (kc - KPG * qb) * 128)
                c = r
                ps = ps_s.tile([128, 2, TCH], FP, name="ps", tag="ps_s")
                for hi in range(2):
                    hb = hi * 64
                    nc.tensor.matmul(
                        ps[:, hi, off:TCH],
                        lhsT=kT[c][hb:hb + 64, kc * 128:(kc + 1) * 128],
                        rhs=qT[c][hb:hb + 64, q0 + off:q0 + TCH],
                        start=True,
                        stop=True,
                    )
                p = p_pool.tile([128, 2, TCH], BF, name="p", tag="p")
                nc.scalar.activation(p[:, :, off:TCH], ps[:, :, off:TCH], EXP, scale=SCALE)
                if kc >= KPG * qb:  # diagonal band: triangular mask
                    # DVE: tiny (bf16 2x) and on the exp->PV critical path;
                    # Pool would queue it behind DMA descriptor generation
                    eng = nc.gpsimd if "poolmask" in opts else nc.vector
                    for hi in range(2):
                        eng.tensor_mul(
                            out=p[:, hi, off:off + 128],
                            in0=p[:, hi, off:off + 128],
                            in1=tri_sb,
                        )
                return p

            pv = {h: ps_v.tile([65, TCH], FP, name=f"ppv{h}", tag="ps_v") for h in hs}

            def pv_mm(kc, p):
                off = max(0, (kc - KPG * qb) * 128)
                for hi, h in enumerate(hs):
                    nc.tensor.matmul(
                        pv[h][:, off:TCH],
                        lhsT=vt[:, kc, 65 * h:65 * h + 65],
                        rhs=p[:, hi, off:TCH],
                        start=(kc == 0),
                        stop=(kc == nkc - 1),
                    )

            pk = {}
            for kc in range(nkc):
                pk[kc] = score_exp(kc)
                if kc >= 2:
                    pv_mm(kc - 2, pk.pop(kc - 2))
                yield
            for kc in (nkc - 2, nkc - 1):
                if kc >= 0:
                    pv_mm(kc, pk.pop(kc))
            # normalize: ctx rows 0..63 per head, denominator row 64.
            def normalize(h, c0, c1, den_act=False):
                hb = (h % 2) * 64
                c = h // 2
                n = c1 - c0
                den = nrm_pool.tile([1, TCH], FP, name="den", tag="den")
                dn = den[:, 0:n]
                if den_act:
                    # tail: stage this head's den on ACT so both heads'
                    # chains overlap (ACT is done with exps by then)
                    nc.scalar.copy(dn, pv[h][64:65, c0:c1])
                else:
                    nc.vector.tensor_copy(out=R(dn), in_=pv[h][64:65, c0:c1])
                rcp = nrm_pool.tile([1, TCH], FP, name="rcp", tag="rcp")
                rp = rcp[:, 0:n]
                nc.vector.reciprocal_approx_fast(out=rp, in_=dn)
                rcb = nrm_pool.tile([64, TCH], FP, name="rcb", tag="rcb")
                rb = rcb[:, 0:n]
                nc.gpsimd.partition_broadcast(rb, rp, channels=64)
                nc.vector.tensor_mul(
                    out=ctxT[c][hb:hb + 64, q0 + c0:q0 + c1],
                    in0=pv[h][0:64, c0:c1], in1=rb,
                )

            if qb == NG - 1:
                # 256-col chunks so proj(3)'s first token blocks start after
                # the first chunk; h1's den staged on ACT to overlap chains
                for ch in range(2):
                    for hi, h in enumerate(hs):
                        normalize(h, ch * 256, (ch + 1) * 256, den_act=(hi == 1))
                    yield
            else:
                for h in hs:
                    normalize(h, 0, TCH)
                    yield

        def proj(qb):
            """Projection for query block qb (4 token blocks of 128), in
            512-col halves so each occupies one filler PSUM bank. Drains go
            to DVE while attention exps overlap; the final block's drain is
            split DVE/ACT with half-stores on two queues to shorten the
            kernel tail."""
            last = qb == NG - 1
            for j in range(KPG):
                tb = KPG * qb + j
                obuf = ob_pool.tile([128, D], BF, name="obuf", tag="ob")
                for hh in range(2):
                    # the last projection runs after the scores pool is done
                    # with attention: alternate with it for a 4-slot po
                    # pipeline (2 rotation slots per pool)
                    use_s = last and (2 * j + hh) % 2 == 1
                    pool = ps_s if use_s else ps_f
                    po = pool.tile([128, 512], FP, name="po",
                                   tag="ps_s" if use_s else "ps_f")
                    for c in range(2):
                        nc.tensor.matmul(
                            po,
                            lhsT=ctxT[c][:, tb * 128:(tb + 1) * 128],
                            rhs=wproj_sb[:, c, hh * 512:(hh + 1) * 512],
                            start=(c == 0),
                            stop=(c == 1),
                        )
                    yield
                    osl = obuf[:, hh * 512:(hh + 1) * 512]
                    if last and j >= KPG - 2:
                        # late blocks: normalize is done -- parallel DVE/ACT
                        # drain halves for minimum latency into the stores
                        nc.vector.tensor_copy(out=osl[:, 0:256], in_=po[:, 0:256])
                        nc.scalar.copy(osl[:, 256:512], po[:, 256:512])
                    elif last:
                        # ACT (exp-free by now): DVE still runs the chunked
                        # normalize gating the later blocks
                        nc.scalar.copy(osl, po)
                    else:
                        # DVE: keeps the exp-critical ACT queue clear
                        nc.vector.tensor_copy(out=osl, in_=po)
                    yield
                t0 = tb * 128
                if last and j == KPG - 1:
                    # piecewise stores, each waiting only its drain piece;
                    # the final piece rides the lower-latency SP HWDGE
                    for pc, eng in enumerate(
                        (nc.sync, nc.gpsimd, nc.gpsimd, nc.sync)
                    ):
                        c0 = pc * 256
                        eng.dma_start(
                            out_d[t0:t0 + 128, c0:c0 + 256], obuf[:, c0:c0 + 256]
                        )
                elif last:
                    nc.sync.dma_start(out_d[t0:t0 + 128, 0:512], obuf[:, 0:512])
                    nc.gpsimd.dma_start(out_d[t0:t0 + 128, 512:D], obuf[:, 512:D])
                else:
                    eng = nc.sync if tb % 2 == 0 else nc.gpsimd
                    eng.dma_start(out_d[t0:t0 + 128, :], obuf)
                yield

        def run(gen):
            for _ in gen:
                pass

        class Chain:
            """A shared stream of filler units pulled at per-phase rates.

            pull_through(idx) exhausts generators 0..idx -- REQUIRED before
            an attention phase that reads their outputs (emission order is
            dependency order in the tile framework: a read emitted before
            its writer sees stale memory)."""

            def __init__(self, gens):
                self.gens = list(gens)
                self.gi = 0
                self.credit = 0.0

            def _step(self):
                while self.gi < len(self.gens):
                    try:
                        next(self.gens[self.gi])
                        return True
                    except StopIteration:
                        self.gi += 1
                return False

            def pull(self, n):
                self.credit += n
                while self.credit >= 1.0 and self._step():
                    self.credit -= 1.0

            def pull_through(self, idx):
                while self.gi <= idx:
                    if not self._step():
                        break

            def drain(self):
                while self._step():
                    pass

        def weave(primary, s1c, rate, proj_gen=None, proj_rate=0.0):
            """Emit attention rounds interleaved with stage1-chain units
            (global, carried across phases) and this phase's proj units."""
            pq = Chain([proj_gen]) if proj_gen is not None else None
            for _ in primary:
                s1c.pull(rate)
                if pq is not None:
                    pq.pull(proj_rate)
            if pq is not None:
                pq.drain()

        def whole():
            if phases == "s1":
                for g in range(NG):
                    run(stage1(g, 0))
                    run(stage1(g, 1))
                return
            run(stage1(0, 0))
            s1c = Chain([
                stage1(0, 1), stage1(1, 0), stage1(1, 1), stage1(2, 0),
                stage1(2, 1), stage1(3, 0), stage1(3, 1),
            ])
            # chain index of s1(qb, half) is 2*qb + half - 1; attention(qb, r)
            # reads qT[r]/kT[r]/vt written by s1(qb, r) from its FIRST round
            # (the query group), so that generator must be exhausted first.
            bulk_consts()
            prefetch(1)
            weave(attention(0, 0), s1c, 2.5)
            s1c.pull_through(0)
            weave(attention(0, 1), s1c, 2.5)
            prefetch(2)
            s1c.pull_through(1)
            weave(attention(1, 0), s1c, 1.0)
            s1c.pull_through(2)
            weave(attention(1, 1), s1c, 1.0, proj(0), 2.0)
            prefetch(3)
            s1c.pull_through(3)
            weave(attention(2, 0), s1c, 1.0)
            s1c.pull_through(4)
            weave(attention(2, 1), s1c, 1.0, proj(1), 1.43)
            s1c.pull_through(5)
            weave(attention(3, 0), s1c, 0.7)
            s1c.pull_through(6)
            weave(attention(3, 1), s1c, 1.0, proj(2), 1.1)
            s1c.drain()
            run(proj(3))
            if dbg:
                for c in range(2):
                    nc.sync.dma_start(qt_dbg[c], qT[c])
                    nc.sync.dma_start(kt_dbg[c], kT[c])
                    nc.sync.dma_start(ctx_dbg[c], ctxT[c])
                nc.sync.dma_start(vt_dbg, vt)

        if loop_n == 1:
            whole()
        else:
            with tc.For_i(0, loop_n, 1):
                whole()

    nc.compile()
    return nc


@functools.lru_cache(maxsize=4)
def _get_program(loop_n=1, phases="all", opts=""):
    return _build_program(loop_n, phases, opts)


def _host_inputs(x, w_qkv, w_proj):
    """Build the 8 per-core input maps from the full problem inputs."""
    from ml_dtypes import bfloat16

    x = np.asarray(x, dtype=np.float32)
    w_qkv = np.asarray(w_qkv, dtype=np.float32)
    w_proj = np.asarray(w_proj, dtype=np.float32)

    # x transposed per batch: (KC, 128, S) bf16
    xt = {
        b: np.ascontiguousarray(x[b].T).reshape(KC, 128, S).astype(bfloat16)
        for b in range(B)
    }

    # RoPE tables, transposed + pair-replicated; sin is sign-folded.
    inv_freq = 1.0 / (ROPE_BASE ** (np.arange(0, Dh, 2, dtype=np.float32) / Dh))
    tpos = np.arange(S, dtype=np.float32)
    freqs = np.outer(tpos, inv_freq)                      # (S, 32)
    emb = np.concatenate([freqs, freqs], axis=-1)         # (S, 64)
    cosT = np.cos(emb).T.astype(np.float32)               # (64, S)
    sinT = np.sin(emb).T.astype(np.float32)
    sinT_f = sinT.copy()
    sinT_f[:32] *= -1.0                                   # fold rotate_half sign
    cos_full = np.ascontiguousarray(np.tile(cosT, (2, 1))).astype(bfloat16)
    sin_full = np.ascontiguousarray(np.tile(sinT_f, (2, 1))).astype(bfloat16)

    r = np.arange(128)
    tri = (r[None, :] >= r[:, None]).astype(bfloat16)     # tri[r, c] = c >= r

    wq = w_qkv[:, 0:D]
    wk = w_qkv[:, D:2 * D]
    wv = w_qkv[:, 2 * D:3 * D]

    in_maps = []
    for c in range(N_CORES):
        b, hq = divmod(c, HPC)
        # wqk half-major: [half, KC, 128, q|k|v x 128]
        wqk_c = np.stack([
            np.concatenate(
                [w[:, np.r_[(4 * hq + 2 * half) * 64:(4 * hq + 2 * half + 2) * 64]]
                 for w in (wq, wk, wv)], axis=1
            ).reshape(KC, 128, 384)
            for half in range(2)
        ])  # (2, KC, 128, 384)
        in_maps.append({
            "xt": xt[b],
            "wqk": np.ascontiguousarray(wqk_c).astype(bfloat16),
            "wproj": np.ascontiguousarray(
                w_proj[4 * hq * 64:(4 * hq + 4) * 64, :]
            ).reshape(2, 128, D).astype(bfloat16),
            "cost": cos_full,
            "sint": sin_full,
            "tri": tri,
            "onesc": np.ones((128, NKB, 65 * HPC), dtype=bfloat16),
            "onesr": np.ones((1, 64), dtype=np.float32),
            "ident": np.eye(128, dtype=bfloat16),
        })
    return in_maps


_last_results = None


def kernel(x, w_qkv, w_proj):
    global _last_results
    from concourse.bass_utils import run_bass_kernel_spmd

    nc = _get_program()
    in_maps = _host_inputs(x, w_qkv, w_proj)
    trace = bool(int(os.environ.get("KERNEL_TRACE", "0")))
    kwargs = {}
    if trace:
        kwargs["trace"] = True
        kwargs["trace_cores"] = list(range(N_CORES))
    res = run_bass_kernel_spmd(nc, in_maps, core_ids=list(range(N_CORES)), **kwargs)
    _last_results = res
    acc = np.zeros((B, S, D), dtype=np.float32)
    for c, r in enumerate(res.results):
        acc[c // HPC] += r["out"].astype(np.float32)
    return acc
